# revision 1
# baseline (speedup 1.0000x reference)
"""Cross-conditional GPT2 sparse attention block on 8 Trainium2 NeuronCores.

Sharding: core = (batch b in 0..3) x (head-group g in 0..1, 6 heads each).

v2 schedule: the whole kernel is emitted as one software-pipelined stream so
the PE never stalls (it needs ~3us of continuous execution to reach its max
p-state):
  - unit (g, h) pipeline: scores(i) -> attv(i-1) -> den-bcast(i-2), with
    projection / output-projection matmul chains interleaved as PE filler
    inside the score phases (which are ACT-paced).
  - diag masks applied as ONE strided 3D multiply per (g, h) against a
    precomputed mask stack (instead of one multiply per j-tile).
  - softmax denominator: reciprocal_approx_fast straight off the PSUM row,
    broadcast across the 64 head-dim partitions with an f32r matmul
    (1 cycle/row at N>=512), then a single fused multiply into yT.
  - q/k/v PSUM->SBUF casts (with bias) run on the Scalar engine, freeing DVE.
  - output partials are written f16; host sums pairs + bp (bv folded into bp
    host-side, exact since softmax rows sum to 1).
"""

import sys

sys.path.insert(0, "/opt/trn_rl_repo")

from collections import deque
from contextlib import ExitStack

import numpy as np

import concourse.bacc as bacc
import concourse.bass as bass
import concourse.mybir as mybir
import concourse.tile as tile
from concourse.bass_utils import run_bass_kernel_spmd

# ---- problem constants (hardcoded per spec) ----
B = 4
T = 512
N = 8
C = 768
NHEAD = 12
L = 3 * T + 4 * N  # 1568
P = 128
G = C // 2  # 384 channels per head-group
NH = 6  # heads per core
D = 64  # head dim
ET = C // P  # 6 e-tiles (contraction of x @ W)
CT = G // P  # 3 c-tiles of the group's channels
NJT = (L + P - 1) // P  # 13 j tiles (12x128 + 32)
SLOT = 544  # pt slot width per j-tile (max interval length)
I_CHUNKS = [(0, 512), (512, 512), (1024, 512), (1536, 32)]
ICH0 = (0, 512, 1024, 1536)
SCALE = 1.0 / 8.0  # 1/sqrt(64)

F32 = mybir.dt.float32
F32R = mybir.dt.float32r
BF16 = mybir.dt.bfloat16
F16 = mybir.dt.float16

_NC = None  # cached compiled Bass program


def _jl(jt):
    return P if jt < NJT - 1 else L - (NJT - 1) * P  # 128 or 32


def _ich_of(a):
    return 3 if a == 1536 else a // 512


# (group) -> per-jt score interval (a, ln).
# g0 = upper rows (i 0..512), jts 0..3; g1 = lower rows; g2 = torso+text rows.
def _grp_interval(g, jt):
    j0 = jt * P
    f0 = (jt % 4) * P if jt <= 11 else 0
    if g == 0:
        return (j0, 512 - j0) if jt <= 3 else None
    if g == 1:
        s = j0 if jt <= 3 else f0
        return (512 + s, 512 - s)
    s = j0 if jt <= 3 else f0
    return (1024 + s, 544 - s)


# diag mask kind per (group, jt in 0..11): 'T1' (tril.T) | 'T2' (strict)
def _grp_diag(g, jt):
    if g == 0:
        return "T1"
    if g == 1:
        return "T1" if jt <= 3 else "T2"
    return "T1" if jt <= 7 else "T2"


_GRP_JTS = {0: list(range(0, 4)), 1: list(range(0, 13)), 2: list(range(0, 13))}
_GRP_ITS = {0: range(0, 4), 1: range(4, 8), 2: range(8, 13)}
# i-chunks whose rows belong to group g (for normalization)
_GRP_ICH = {0: [0], 1: [1], 2: [2, 3]}


def _build_program():
    nc = bacc.Bacc("TRN2", target_bir_lowering=False, debug=False)

    xT_d = nc.dram_tensor("xT", [C, L], F16, kind="ExternalInput")
    wq_d = nc.dram_tensor("wqT", [C, G], F16, kind="ExternalInput")
    wk_d = nc.dram_tensor("wkT", [C, G], F16, kind="ExternalInput")
    wv_d = nc.dram_tensor("wvT", [C, G], F16, kind="ExternalInput")
    wp_d = nc.dram_tensor("wpT", [G, C], F16, kind="ExternalInput")
    bq_d = nc.dram_tensor("bqP", [P, CT], F32, kind="ExternalInput")
    bk_d = nc.dram_tensor("bkP", [P, CT], F32, kind="ExternalInput")
    mstk_d = nc.dram_tensor("maskStk", [P, 28 * P], F16, kind="ExternalInput")
    maskt_d = nc.dram_tensor("maskTxt", [32, 1024], F16, kind="ExternalInput")
    out_d = nc.dram_tensor("out_part", [L, C], F16, kind="ExternalOutput")

    # mask-stack slot offset per group (g0: 4 slots, g1: 12, g2: 12)
    MOFF = {0: 0, 1: 4, 2: 16}

    with tile.TileContext(nc) as tc, ExitStack() as big:
        persist = big.enter_context(tc.tile_pool(name="persist", bufs=1))
        phA = big.enter_context(tc.tile_pool(name="phA", bufs=1))
        phB = big.enter_context(tc.tile_pool(name="phB", bufs=1))
        psS = big.enter_context(tc.tile_pool(name="psS", bufs=4, space="PSUM"))
        psY = big.enter_context(tc.tile_pool(name="psY", bufs=4, space="PSUM"))

        # persistent SBUF tensors
        qT = persist.tile([P, CT, L], F16, name="qT")
        kT = persist.tile([P, CT, L], F16, name="kT")
        v_ones = persist.tile([P, NJT, NH, D + 1], F16, name="v_ones")
        maskStk = persist.tile([P, 28, P], F16, name="maskStk_sb")
        maskTx = persist.tile([32, 1024], F16, name="maskTx_sb")
        yT = persist.tile([P, CT, L], F16, name="yT")
        wp_sb = persist.tile([P, CT, C], F16, name="wp_sb")

        nc.sync.dma_start(maskStk[:], mstk_d.rearrange("p (s c) -> p s c", c=P))
        nc.sync.dma_start(maskTx[:], maskt_d[:])
        nc.sync.dma_start(wp_sb[:], wp_d.rearrange("(ct p) n -> p ct n", p=P))
        nc.gpsimd.memset(v_ones[:], 1.0)

        # ---------- Phase A tiles + input DMA ----------
        xT = phA.tile([P, ET, L], F16, name="xT_sb")
        wq_sb = phA.tile([P, ET, G], F16, name="wq_sb")
        wk_sb = phA.tile([P, ET, G], F16, name="wk_sb")
        wv_sb = phA.tile([P, ET, G], F16, name="wv_sb")
        bq_sb = phA.tile([P, CT], F32, name="bq_sb")
        bk_sb = phA.tile([P, CT], F32, name="bk_sb")

        for et in range(ET):
            nc.sync.dma_start(xT[:, et, :], xT_d[et * P : (et + 1) * P, :])
        for w_sb, w_d in ((wq_sb, wq_d), (wk_sb, wk_d), (wv_sb, wv_d)):
            for et in range(ET):
                nc.sync.dma_start(w_sb[:, et, :], w_d[et * P : (et + 1) * P, :])
        nc.sync.dma_start(bq_sb[:], bq_d[:])
        nc.sync.dma_start(bk_sb[:], bk_d[:])

        # ---------- projection chain emitters (PE + ACT cast) ----------
        def emit_qk_chain(dst, w_sb, b_sb, ct, ich):
            i0, ilen = I_CHUNKS[ich]
            ps = psS.tile([P, 512], F32, name="ps_qk", tag="ps_s")
            for et in range(ET):
                nc.tensor.matmul(
                    ps[:, :ilen],
                    w_sb[:, et, ct * P : (ct + 1) * P],
                    xT[:, et, i0 : i0 + ilen],
                    start=(et == 0),
                    stop=(et == ET - 1),
                    skip_group_check=True,
                )
            nc.vector.tensor_scalar(
                dst[:, ct, i0 : i0 + ilen],
                ps[:, :ilen],
                b_sb[:, ct : ct + 1],
                None,
                mybir.AluOpType.add,
            )

        def emit_v_chain(it):
            il = _jl(it)
            ps = psS.tile([P, 512], F32, name="ps_v", tag="ps_s")
            for et in range(ET):
                nc.tensor.matmul(
                    ps[:il, :G],
                    xT[:, et, it * P : it * P + il],
                    wv_sb[:, et, :],
                    start=(et == 0),
                    stop=(et == ET - 1),
                    skip_group_check=True,
                )
            nc.vector.tensor_copy(
                v_ones[:il, it, :, 0:D],
                ps[:il, :G].rearrange("p (h d) -> p h d", h=NH),
            )

        def emit_outproj_chain(it, nch):
            il = _jl(it)
            ps_o = psS.tile([P, 512], F32, name="ps_o", tag="ps_s")
            for kt in range(CT):
                nc.tensor.matmul(
                    ps_o[:il, :G],
                    yT[:, kt, it * P : it * P + il],
                    wp_sb[:, kt, nch * G : (nch + 1) * G],
                    start=(kt == 0),
                    stop=(kt == CT - 1),
                    skip_group_check=True,
                )
            o_sb = phB.tile([P, G], F16, name="o_sb", tag="o_sb", bufs=3)
            nc.vector.tensor_copy(o_sb[:il, :], ps_o[:il, :G])
            nc.sync.dma_start(
                out_d[it * P : it * P + il, nch * G : (nch + 1) * G], o_sb[:il, :]
            )

        # ---------- attention unit emitters ----------
        units = [(g, h) for g in range(3) for h in range(NH)]
        urec = [dict() for _ in units]  # per-unit state (pt tile, chunks, psy)

        fillers = deque()

        def pop_filler(n=1):
            for _ in range(n):
                if fillers:
                    fillers.popleft()()

        def emit_scores(i):
            g, h = units[i]
            pof = D * (h % 2)
            ct = h // 2
            pt = phB.tile([P, NJT, SLOT], F16, name="pt", tag="pt", bufs=3)
            chunks = []  # (jt, slot, ca, cl, slot_off)
            cols = 0
            for slot, jt in enumerate(_GRP_JTS[g]):
                iv = _grp_interval(g, jt)
                jl = _jl(jt)
                a, ln = iv
                cparts = [(a, min(ln, 512))]
                if ln > 512:
                    cparts.append((a + 512, ln - 512))
                for ca, cl in cparts:
                    ps_s = psS.tile([P, 512], F32, name="ps_s", tag="ps_s")
                    nc.tensor.matmul(
                        ps_s[:jl, :cl],
                        kT[pof : pof + D, ct, jt * P : jt * P + jl],
                        qT[pof : pof + D, ct, ca : ca + cl],
                        start=True,
                        stop=True,
                        skip_group_check=True,
                    )
                    nc.scalar.activation(
                        pt[:jl, slot, ca - a : ca - a + cl],
                        ps_s[:jl, :cl],
                        mybir.ActivationFunctionType.Exp,
                        bias=0.0,
                        scale=SCALE,
                    )
                    chunks.append((jt, slot, ca, cl, ca - a))
                    cols += cl
                    if cols >= 1024:
                        cols -= 1024
                        pop_filler()
            urec[i]["pt"] = pt
            urec[i]["chunks"] = chunks

        def emit_mask(i):
            g, h = units[i]
            pt = urec[i]["pt"]
            ndiag = 4 if g == 0 else 12
            # 4-slot windows so att@v on early j-tiles only waits its own
            # window's exps, not the whole head's
            for s0 in range(0, ndiag, 4):
                nc.vector.tensor_tensor(
                    pt[:, s0 : s0 + 4, 0:P],
                    pt[:, s0 : s0 + 4, 0:P],
                    maskStk[:, MOFF[g] + s0 : MOFF[g] + s0 + 4, :],
                    mybir.AluOpType.mult,
                )
            if g >= 1:
                m0 = 0 if g == 1 else 512
                nc.vector.tensor_tensor(
                    pt[0:32, 12, 0:512],
                    pt[0:32, 12, 0:512],
                    maskTx[0:32, m0 : m0 + 512],
                    mybir.AluOpType.mult,
                )

        def emit_attv(i):
            g, h = units[i]
            pt = urec[i]["pt"]
            ps_y = {}
            started = set()
            last_jt = _GRP_JTS[g][-1]
            for jt, slot, ca, cl, soff in urec[i]["chunks"]:
                jl = _jl(jt)
                parts = [(ca, cl, soff)]
                if ca < 1536 < ca + cl:
                    parts = [
                        (ca, 1536 - ca, soff),
                        (1536, ca + cl - 1536, soff + 1536 - ca),
                    ]
                for pa, pl, poff in parts:
                    ich = _ich_of(pa)
                    off = pa - ICH0[ich]
                    if ich not in ps_y:
                        ps_y[ich] = psY.tile(
                            [D + 1, 512], F32, name=f"ps_y{ich}", tag="ps_y"
                        )
                    nc.tensor.matmul(
                        ps_y[ich][:, off : off + pl],
                        v_ones[:jl, jt, h, :],
                        pt[:jl, slot, poff : poff + pl],
                        start=ich not in started,
                        stop=(jt == last_jt),
                        skip_group_check=True,
                    )
                    started.add(ich)
            urec[i]["ps_y"] = ps_y

        def emit_den_copy(i):
            # pull the denominator row (PSUM partition 64) into SBUF partition
            # 0 on the (idle) GpSimd engine; custom DVE ops can't read a
            # nonzero base partition and PSUM APs must be 32-aligned.
            dens = []
            for ich, psy in urec[i]["ps_y"].items():
                ilen = I_CHUNKS[ich][1]
                den = phB.tile([1, 512], F32, name="den", tag="den", bufs=4)
                nc.vector.tensor_copy(den[0:1, :ilen], psy[D : D + 1, :ilen])
                dens.append((ich, psy, den))
            urec[i]["dens"] = dens

        def emit_norm_recip(i):
            # reciprocal on DVE, then broadcast across the 64 head-dim
            # partitions on GpSimd. The consuming multiply runs later in the
            # same loop so GpSimd latency never stalls DVE/PE.
            recs = []
            for ich, psy, den in urec[i]["dens"]:
                ilen = I_CHUNKS[ich][1]
                rc = phB.tile([1, 512], F32, name="rc", tag="rc", bufs=4)
                nc.vector.reciprocal_approx_fast(
                    out=rc[0:1, :ilen], in_=den[0:1, :ilen]
                )
                rc_bc = phB.tile([D, 512], F32, name="rc_bc", tag="rc_bc", bufs=4)
                nc.gpsimd.partition_broadcast(rc_bc[:, :ilen], rc[0:1, :ilen])
                recs.append((ich, psy, rc_bc))
            urec[i]["recs"] = recs

        def emit_norm_mult(i):
            g, h = units[i]
            pof = D * (h % 2)
            ct = h // 2
            for ich, psy, rc_bc in urec[i]["recs"]:
                i0, ilen = I_CHUNKS[ich]
                nc.vector.tensor_tensor(
                    yT[pof : pof + D, ct, i0 : i0 + ilen],
                    psy[0:D, :ilen],
                    rc_bc[:, :ilen],
                    mybir.AluOpType.mult,
                )

        # ---------- upfront: phase A for g0's needs ----------
        for ct in range(CT):
            emit_qk_chain(kT, wk_sb, bk_sb, ct, 0)
            emit_qk_chain(qT, wq_sb, bq_sb, ct, 0)
        for it in range(4):
            emit_v_chain(it)

        # filler groups consumed during g0 / g1 / g2
        # g1/g2 attend ALL key tiles, so every kT chunk and every v tile must
        # be emitted before g1's first score/attv; only qT splits by row group.
        fill_g0 = []
        for ich in (1, 2, 3):
            for ct in range(CT):
                fill_g0.append(
                    lambda ct=ct, ich=ich: emit_qk_chain(kT, wk_sb, bk_sb, ct, ich)
                )
        for ct in range(CT):
            fill_g0.append(lambda ct=ct: emit_qk_chain(qT, wq_sb, bq_sb, ct, 1))
        for it in range(4, NJT):
            fill_g0.append(lambda it=it: emit_v_chain(it))

        fill_g1 = []
        for ich in (2, 3):
            for ct in range(CT):
                fill_g1.append(
                    lambda ct=ct, ich=ich: emit_qk_chain(qT, wq_sb, bq_sb, ct, ich)
                )

        FILL = {0: fill_g0, 1: fill_g1, 2: []}

        # ---------- main software-pipelined loop ----------
        prev_g = None
        for i, (g, h) in enumerate(units):
            if g != prev_g:
                # drain leftovers of the previous phase's fillers (their
                # outputs gate this group's first scores), then load new ones
                while fillers:
                    fillers.popleft()()
                fillers.extend(FILL[g])
                prev_g = g
                if g == 2:
                    # out-projection of g0 becomes filler now (its yT rows
                    # finished at norm_mult(g0, h5) which was emitted at i-3)
                    for it in _GRP_ITS[0]:
                        for nch in range(2):
                            fillers.append(
                                lambda it=it, nch=nch: emit_outproj_chain(it, nch)
                            )
            if i == 15:
                # g1's yT finished at norm_mult(unit 11) emitted at i=14
                for it in _GRP_ITS[1]:
                    for nch in range(2):
                        fillers.append(
                            lambda it=it, nch=nch: emit_outproj_chain(it, nch)
                        )
            if i >= 2:
                emit_norm_recip(i - 2)
            emit_scores(i)
            emit_mask(i)
            if i >= 1:
                emit_attv(i - 1)
                emit_den_copy(i - 1)
            if i >= 2:
                emit_norm_mult(i - 2)
            pop_filler()

        # ---------- tail ----------
        nunits = len(units)
        emit_attv(nunits - 1)
        emit_den_copy(nunits - 1)
        for i in (nunits - 2, nunits - 1):
            emit_norm_recip(i)
            emit_norm_mult(i)
        while fillers:
            fillers.popleft()()
        for it in _GRP_ITS[2]:
            for nch in range(2):
                emit_outproj_chain(it, nch)

    nc.compile()
    return nc


def _build_mask_np(seg_starts, seg_ends):
    """True = masked. Mirrors reference._build_mask in numpy."""
    ML = 3 * T
    tril = np.tril(np.ones((T, T), dtype=bool))
    sl = np.tril(np.ones((T, T), dtype=bool), -1)
    m = np.zeros((L, L), dtype=bool)
    m[:ML, :ML] = True
    m[0:T, 0:T] = ~tril
    m[T : 2 * T, 0:T] = ~tril
    m[T : 2 * T, T : 2 * T] = ~sl
    m[T : 2 * T, 2 * T : 3 * T] = ~sl
    m[2 * T : 3 * T, 0:T] = ~tril
    m[2 * T : 3 * T, T : 2 * T] = ~tril
    m[2 * T : 3 * T, 2 * T : 3 * T] = ~sl
    m[:ML, ML:] = True
    frames = np.arange(T)[None, :, None]
    allowed = (frames >= seg_starts[:, None, :]) & (frames < seg_ends[:, None, :])
    mask = np.broadcast_to(m[None], (B, L, L)).copy()
    for row0, col_blocks in ((T, (0, 2, 3)), (2 * T, (1, 2, 3))):
        for j in col_blocks:
            c0 = ML + j * N
            mask[:, row0 : row0 + T, c0 : c0 + N] &= ~allowed
    return mask


def get_nc():
    global _NC
    if _NC is None:
        _NC = _build_program()
    return _NC


def _build_maskstk():
    r = np.arange(P)
    t1 = (r[:, None] <= r[None, :]).astype(np.float16)  # tril.T
    t2 = (r[:, None] < r[None, :]).astype(np.float16)  # strict
    stk = np.empty((P, 28, P), dtype=np.float16)
    s = 0
    for g in range(3):
        for jt in _GRP_JTS[g][: (4 if g == 0 else 12)]:
            stk[:, s, :] = t1 if _grp_diag(g, jt) == "T1" else t2
            s += 1
    assert s == 28
    return stk.reshape(P, 28 * P)


def make_in_maps(x, Wq, bq, Wk, bk, Wv, bv, Wp, bp, seg_starts, seg_ends):
    mask = _build_mask_np(np.asarray(seg_starts), np.asarray(seg_ends))
    maskstk = _build_maskstk()
    in_maps = []
    for core in range(8):
        b, g = core // 2, core % 2
        gs = slice(g * G, (g + 1) * G)
        allowT = ~mask[b].T  # [j, i]
        maskTx = np.ascontiguousarray(
            allowT[1536:1568, 512:1536].astype(np.float16)
        )
        in_maps.append(
            {
                "xT": np.ascontiguousarray(x[b].T).astype(np.float16),
                "wqT": np.ascontiguousarray(Wq[gs, :].T).astype(np.float16),
                "wkT": np.ascontiguousarray(Wk[gs, :].T).astype(np.float16),
                "wvT": np.ascontiguousarray(Wv[gs, :].T).astype(np.float16),
                "wpT": np.ascontiguousarray(Wp[:, gs].T).astype(np.float16),
                "bqP": np.ascontiguousarray(bq[gs].reshape(CT, P).T),
                "bkP": np.ascontiguousarray(bk[gs].reshape(CT, P).T),
                "maskStk": maskstk,
                "maskTxt": maskTx,
            }
        )
    return in_maps


def kernel(x, Wq, bq, Wk, bk, Wv, bv, Wp, bp, seg_starts, seg_ends, T_motion=None,
           N=None, _trace=False, **_unused):
    x = np.asarray(x, np.float32)
    args = [np.asarray(a, np.float32) for a in (Wq, bq, Wk, bk, Wv, bv, Wp, bp)]
    Wq, bq, Wk, bk, Wv, bv, Wp, bp = args
    nc = get_nc()
    in_maps = make_in_maps(x, Wq, bq, Wk, bk, Wv, bv, Wp, bp, seg_starts, seg_ends)
    res = run_bass_kernel_spmd(nc, in_maps, core_ids=list(range(8)), trace=_trace)
    parts = [np.asarray(r["out_part"], np.float32) for r in res.results]
    # v-bias folds into the output bias exactly: att rows sum to 1, so
    # y = att@(v+bv) = att@v + bv, and (y+bv)@Wp.T = y@Wp.T + bv@Wp.T
    bp_eff = bp + bv @ Wp.T
    y = np.empty((B, L, C), np.float32)
    for b in range(B):
        y[b] = parts[2 * b] + parts[2 * b + 1] + bp_eff
    if _trace:
        kernel.last_results = res
    return y



# revision 4
# speedup vs baseline: 1.0297x; 1.0297x over previous
"""Cross-conditional GPT2 sparse attention block on 8 Trainium2 NeuronCores.

Sharding: core = (batch b in 0..3) x (head-group g in 0..1, 6 heads each).

v3 schedule: one globally software-pipelined PE stream designed around the
TRN2 p-state rule (PE reaches 2.4 GHz only after ~3us of gap-free execution;
any stall drops it to 1.2 GHz):
  - projection chains (q/k/v/out-proj) are spread across the WHOLE kernel as
    PE filler, scheduled by dependency deadline (k/v before g1's first use,
    q-ich2/3 before g2, out-proj released as each row-group's yT completes),
    instead of front-loaded - so the attention phase always has independent
    PE work between dependent score matmuls.
  - attv chunks of unit i-1 are interleaved 1:1 with scores chunks of unit i.
  - separate PSUM pools (scores 3 banks / attv 3 / projections 2) so chains
    never cross-stall on another chain's drain.
  - softmax denominator: ones-COLUMN at v_ones[...,0] puts the den row at
    PSUM partition 0, so DVE reciprocal_approx_fast reads it directly
    (no [1,512] den copy); y rows live at partitions 32..96 (PSUM APs must
    be 32-aligned); GpSimd broadcasts 1/den, DVE multiplies into yT.
  - exp on ACT; masks/casts/recip/norm on DVE; broadcasts on GpSimd (GpSimd
    cannot access PSUM, so all psum->sbuf drains stay on DVE).
"""

import sys

sys.path.insert(0, "/opt/trn_rl_repo")

from collections import deque
from contextlib import ExitStack

import numpy as np

import concourse.bacc as bacc
import concourse.bass as bass
import concourse.mybir as mybir
import concourse.tile as tile
from concourse.bass_utils import run_bass_kernel_spmd

# ---- problem constants (hardcoded per spec) ----
B = 4
T = 512
N = 8
C = 768
NHEAD = 12
L = 3 * T + 4 * N  # 1568
P = 128
G = C // 2  # 384 channels per head-group
NH = 6  # heads per core
D = 64  # head dim
ET = C // P  # 6 e-tiles (contraction of x @ W)
CT = G // P  # 3 c-tiles of the group's channels
NJT = (L + P - 1) // P  # 13 j tiles (12x128 + 32)
SLOT = 544  # pt slot width per j-tile (max interval length)
I_CHUNKS = [(0, 512), (512, 512), (1024, 512), (1536, 32)]
ICH0 = (0, 512, 1024, 1536)
SCALE = 1.0 / 8.0  # 1/sqrt(64)
V0 = 64  # v rows base partition in psY (ones/den row at partition 0;
# PSUM APs must not cross a 64-partition boundary unless 64-aligned)
VW = V0 + D  # v_ones width 128: [0]=ones, [64:128]=v

F32 = mybir.dt.float32
F16 = mybir.dt.float16

_NC = None  # cached compiled Bass program


def _jl(jt):
    return P if jt < NJT - 1 else L - (NJT - 1) * P  # 128 or 32


def _ich_of(a):
    return 3 if a == 1536 else a // 512


# (group) -> per-jt score interval (a, ln).
# g0 = upper rows (i 0..512), jts 0..3; g1 = lower rows; g2 = torso+text rows.
def _grp_interval(g, jt):
    j0 = jt * P
    f0 = (jt % 4) * P if jt <= 11 else 0
    if g == 0:
        return (j0, 512 - j0) if jt <= 3 else None
    if g == 1:
        s = j0 if jt <= 3 else f0
        return (512 + s, 512 - s)
    s = j0 if jt <= 3 else f0
    return (1024 + s, 544 - s)


# diag mask kind per (group, jt in 0..11): 'T1' (tril.T) | 'T2' (strict)
def _grp_diag(g, jt):
    if g == 0:
        return "T1"
    if g == 1:
        return "T1" if jt <= 3 else "T2"
    return "T1" if jt <= 7 else "T2"


_GRP_JTS = {0: list(range(0, 4)), 1: list(range(0, 13)), 2: list(range(0, 13))}
_GRP_ITS = {0: range(0, 4), 1: range(4, 8), 2: range(8, 13)}


def _unit_chunks(g):
    """Score chunks for a unit: (jt, slot, ca, cl, soff), slot order."""
    chunks = []
    for slot, jt in enumerate(_GRP_JTS[g]):
        a, ln = _grp_interval(g, jt)
        cparts = [(a, min(ln, 512))]
        if ln > 512:
            cparts.append((a + 512, ln - 512))
        for ca, cl in cparts:
            chunks.append((jt, slot, ca, cl, ca - a))
    return chunks


def _build_program():
    nc = bacc.Bacc("TRN2", target_bir_lowering=False, debug=False)

    xT_d = nc.dram_tensor("xT", [C, L], F16, kind="ExternalInput")
    wq_d = nc.dram_tensor("wqT", [C, G], F16, kind="ExternalInput")
    wk_d = nc.dram_tensor("wkT", [C, G], F16, kind="ExternalInput")
    wv_d = nc.dram_tensor("wvT", [C, G], F16, kind="ExternalInput")
    wp_d = nc.dram_tensor("wpT", [G, C], F16, kind="ExternalInput")
    bq_d = nc.dram_tensor("bqP", [P, CT], F32, kind="ExternalInput")
    bk_d = nc.dram_tensor("bkP", [P, CT], F32, kind="ExternalInput")
    mstk_d = nc.dram_tensor("maskStk", [P, 28 * P], F16, kind="ExternalInput")
    maskt_d = nc.dram_tensor("maskTxt", [32, 1024], F16, kind="ExternalInput")
    out_d = nc.dram_tensor("out_part", [L, C], F16, kind="ExternalOutput")

    # mask-stack slot offset per group (g0: 4 slots, g1: 12, g2: 12)
    MOFF = {0: 0, 1: 4, 2: 16}
    units = [(g, h) for g in range(3) for h in range(NH)]

    with tile.TileContext(nc) as tc, ExitStack() as big:
        persist = big.enter_context(tc.tile_pool(name="persist", bufs=1))
        phA = big.enter_context(tc.tile_pool(name="phA", bufs=1))
        phB = big.enter_context(tc.tile_pool(name="phB", bufs=1))
        psS = big.enter_context(tc.tile_pool(name="psS", bufs=3, space="PSUM"))
        psYp = big.enter_context(tc.tile_pool(name="psYp", bufs=3, space="PSUM"))
        psP = big.enter_context(tc.tile_pool(name="psP", bufs=2, space="PSUM"))

        # persistent SBUF tensors
        qT = persist.tile([P, CT, L], F16, name="qT")
        kT = persist.tile([P, CT, L], F16, name="kT")
        v_ones = persist.tile([P, NJT, NH, VW], F16, name="v_ones")
        maskStk = persist.tile([P, 28, P], F16, name="maskStk_sb")
        maskTx = persist.tile([32, 1024], F16, name="maskTx_sb")
        yT = persist.tile([P, CT, L], F16, name="yT")
        wp_sb = persist.tile([P, CT, C], F16, name="wp_sb")

        nc.sync.dma_start(maskStk[:], mstk_d.rearrange("p (s c) -> p s c", c=P))
        nc.sync.dma_start(maskTx[:], maskt_d[:])
        nc.sync.dma_start(wp_sb[:], wp_d.rearrange("(ct p) n -> p ct n", p=P))
        # ones column at free index 0 (-> den at PSUM partition 0); cols
        # 1..63 stay 1.0 but psY rows 1..63 are never read.
        nc.gpsimd.memset(v_ones[:], 1.0)

        # ---------- input tiles + DMA (x split by i-chunk: first chains
        # only wait on their own chunk) ----------
        xT = phA.tile([P, ET, L], F16, name="xT_sb")
        wq_sb = phA.tile([P, ET, G], F16, name="wq_sb")
        wk_sb = phA.tile([P, ET, G], F16, name="wk_sb")
        wv_sb = phA.tile([P, ET, G], F16, name="wv_sb")
        bq_sb = phA.tile([P, CT], F32, name="bq_sb")
        bk_sb = phA.tile([P, CT], F32, name="bk_sb")

        for i0, ilen in I_CHUNKS:
            for et in range(ET):
                nc.sync.dma_start(
                    xT[:, et, i0 : i0 + ilen],
                    xT_d[et * P : (et + 1) * P, i0 : i0 + ilen],
                )
        for w_sb, w_d in ((wk_sb, wk_d), (wq_sb, wq_d), (wv_sb, wv_d)):
            for et in range(ET):
                nc.sync.dma_start(w_sb[:, et, :], w_d[et * P : (et + 1) * P, :])
        nc.sync.dma_start(bq_sb[:], bq_d[:])
        nc.sync.dma_start(bk_sb[:], bk_d[:])

        # ---------- projection chain emitters ----------
        def emit_qk_chain(dst, w_sb, b_sb, ct, ich):
            i0, ilen = I_CHUNKS[ich]
            ps = psP.tile([P, 512], F32, name="ps_p", tag="ps_p")
            for et in range(ET):
                nc.tensor.matmul(
                    ps[:, :ilen],
                    w_sb[:, et, ct * P : (ct + 1) * P],
                    xT[:, et, i0 : i0 + ilen],
                    start=(et == 0),
                    stop=(et == ET - 1),
                    skip_group_check=True,
                )
            nc.vector.tensor_scalar(
                dst[:, ct, i0 : i0 + ilen],
                ps[:, :ilen],
                b_sb[:, ct : ct + 1],
                None,
                mybir.AluOpType.add,
            )

        def emit_v_chain(it):
            il = _jl(it)
            ps = psP.tile([P, 512], F32, name="ps_pv", tag="ps_p")
            for et in range(ET):
                nc.tensor.matmul(
                    ps[:il, :G],
                    xT[:, et, it * P : it * P + il],
                    wv_sb[:, et, :],
                    start=(et == 0),
                    stop=(et == ET - 1),
                    skip_group_check=True,
                )
            nc.vector.tensor_copy(
                v_ones[:il, it, :, V0 : V0 + D],
                ps[:il, :G].rearrange("p (h d) -> p h d", h=NH),
            )

        def emit_outproj_chain(it, nch):
            il = _jl(it)
            ps_o = psP.tile([P, 512], F32, name="ps_po", tag="ps_p")
            for kt in range(CT):
                nc.tensor.matmul(
                    ps_o[:il, :G],
                    yT[:, kt, it * P : it * P + il],
                    wp_sb[:, kt, nch * G : (nch + 1) * G],
                    start=(kt == 0),
                    stop=(kt == CT - 1),
                    skip_group_check=True,
                )
            o_sb = phB.tile([P, G], F16, name="o_sb", tag="o_sb", bufs=3)
            nc.vector.tensor_copy(o_sb[:il, :], ps_o[:il, :G])
            nc.sync.dma_start(
                out_d[it * P : it * P + il, nch * G : (nch + 1) * G], o_sb[:il, :]
            )

        # ---------- attention emitters ----------
        urec = [dict() for _ in units]

        def emit_score_chunk(i, chunk):
            g, h = units[i]
            pof = D * (h % 2)
            ct = h // 2
            jt, slot, ca, cl, soff = chunk
            jl = _jl(jt)
            pt = urec[i]["pt"]
            ps_s = psS.tile([P, 512], F32, name="ps_s", tag="ps_s")
            nc.tensor.matmul(
                ps_s[:jl, :cl],
                kT[pof : pof + D, ct, jt * P : jt * P + jl],
                qT[pof : pof + D, ct, ca : ca + cl],
                start=True,
                stop=True,
                skip_group_check=True,
            )
            nc.scalar.activation(
                pt[:jl, slot, soff : soff + cl],
                ps_s[:jl, :cl],
                mybir.ActivationFunctionType.Exp,
                bias=0.0,
                scale=SCALE,
            )

        def emit_mask_window(i, w):
            g, h = units[i]
            pt = urec[i]["pt"]
            s0 = 4 * w
            nc.vector.tensor_tensor(
                pt[:, s0 : s0 + 4, 0:P],
                pt[:, s0 : s0 + 4, 0:P],
                maskStk[:, MOFF[g] + s0 : MOFF[g] + s0 + 4, :],
                mybir.AluOpType.mult,
            )

        def emit_text_mask(i):
            g, h = units[i]
            pt = urec[i]["pt"]
            m0 = 0 if g == 1 else 512
            nc.vector.tensor_tensor(
                pt[0:32, 12, 0:512],
                pt[0:32, 12, 0:512],
                maskTx[0:32, m0 : m0 + 512],
                mybir.AluOpType.mult,
            )

        def attv_part_list(i):
            g, h = units[i]
            parts = []
            started = set()
            last_jt = _GRP_JTS[g][-1]
            for jt, slot, ca, cl, soff in urec[i]["chunks"]:
                subs = [(ca, cl, soff)]
                if ca < 1536 < ca + cl:
                    subs = [
                        (ca, 1536 - ca, soff),
                        (1536, ca + cl - 1536, soff + 1536 - ca),
                    ]
                for pa, pl, poff in subs:
                    ich = _ich_of(pa)
                    first = ich not in started
                    started.add(ich)
                    parts.append((jt, slot, pa, pl, poff, ich, first, jt == last_jt))
            return parts

        def emit_attv_part(i, part):
            g, h = units[i]
            jt, slot, pa, pl, poff, ich, first, stop = part
            jl = _jl(jt)
            ysd = urec[i].setdefault("ps_y", {})
            if ich not in ysd:
                ysd[ich] = psYp.tile([VW, 512], F32, name="ps_y", tag="ps_y")
            off = pa - ICH0[ich]
            nc.tensor.matmul(
                ysd[ich][:, off : off + pl],
                v_ones[:jl, jt, h, :],
                urec[i]["pt"][:jl, slot, poff : poff + pl],
                start=first,
                stop=stop,
                skip_group_check=True,
            )

        def emit_recip_bcast(i):
            # 1/den straight off PSUM partition 0 (the ones row), then
            # broadcast across the 64 head-dim partitions on GpSimd.
            recs = []
            for ich, psy in urec[i]["ps_y"].items():
                ilen = I_CHUNKS[ich][1]
                rc = phB.tile([1, 512], F32, name="rc", tag="rc", bufs=4)
                nc.vector.reciprocal_approx_fast(
                    out=rc[0:1, :ilen], in_=psy[0:1, :ilen]
                )
                rc_bc = phB.tile([D, 512], F32, name="rc_bc", tag="rc_bc", bufs=4)
                nc.gpsimd.partition_broadcast(rc_bc[:, :ilen], rc[0:1, :ilen])
                recs.append((ich, psy, rc_bc))
            urec[i]["recs"] = recs

        def emit_norm_mult(i):
            g, h = units[i]
            pof = D * (h % 2)
            ct = h // 2
            for ich, psy, rc_bc in urec[i]["recs"]:
                i0, ilen = I_CHUNKS[ich]
                nc.vector.tensor_tensor(
                    yT[pof : pof + D, ct, i0 : i0 + ilen],
                    psy[V0 : V0 + D, :ilen],
                    rc_bc[:, :ilen],
                    mybir.AluOpType.mult,
                )

        # ---------- filler segments (release block, deque) ----------
        # F1: everything g1 needs (+ its attv), consumed during g0 blocks.
        f1 = deque()
        for ct in range(CT):
            if ct == 0:
                f1.append(lambda: emit_qk_chain(qT, wq_sb, bq_sb, 0, 1))
                for ich in (1, 2, 3):
                    f1.append(
                        lambda ich=ich: emit_qk_chain(kT, wk_sb, bk_sb, 0, ich)
                    )
                for it in range(4, NJT):
                    f1.append(lambda it=it: emit_v_chain(it))
            else:
                f1.append(lambda ct=ct: emit_qk_chain(qT, wq_sb, bq_sb, ct, 1))
                for ich in (1, 2, 3):
                    f1.append(
                        lambda ct=ct, ich=ich: emit_qk_chain(
                            kT, wk_sb, bk_sb, ct, ich
                        )
                    )
        # F2: q rows for g2.
        f2 = deque()
        for ich in (2, 3):
            for ct in range(CT):
                f2.append(
                    lambda ct=ct, ich=ich: emit_qk_chain(qT, wq_sb, bq_sb, ct, ich)
                )
        # F3/F4/F5: out-projections, released as each row-group's yT lands.
        f3 = deque(
            (lambda it=it, nch=nch: emit_outproj_chain(it, nch))
            for it in _GRP_ITS[0]
            for nch in range(2)
        )
        f4 = deque(
            (lambda it=it, nch=nch: emit_outproj_chain(it, nch))
            for it in _GRP_ITS[1]
            for nch in range(2)
        )
        f5 = deque(
            (lambda it=it, nch=nch: emit_outproj_chain(it, nch))
            for it in _GRP_ITS[2]
            for nch in range(2)
        )
        segments = [(0, f1), (6, f2), (7, f3), (13, f4), (99, f5)]

        def pop_filler(block):
            for rel, dq in segments:
                if rel <= block and dq:
                    dq.popleft()()
                    return True
            return False

        # per-block filler quotas (g0 blocks absorb all of F1)
        QUOTA = [4, 4, 4, 3, 3, 3, 2, 3, 3, 3, 2, 2, 2, 2, 2, 2, 2, 2]

        # ---------- upfront: phase A for g0's needs ----------
        for ct in range(CT):
            emit_qk_chain(kT, wk_sb, bk_sb, ct, 0)
            emit_qk_chain(qT, wq_sb, bq_sb, ct, 0)
        for it in range(4):
            emit_v_chain(it)

        # ---------- main software-pipelined loop ----------
        for i, (g, h) in enumerate(units):
            if i == 6:
                while f1:
                    f1.popleft()()
            if i == 12:
                while f2:
                    f2.popleft()()
            if i >= 2:
                emit_norm_mult(i - 2)
            urec[i]["pt"] = phB.tile(
                [P, NJT, SLOT], F16, name="pt", tag="pt", bufs=3
            )
            chunks = _unit_chunks(g)
            urec[i]["chunks"] = chunks
            av = attv_part_list(i - 1) if i >= 1 else []
            nS = len(chunks)
            nwin = (4 if g == 0 else 12) // 4
            quota = QUOTA[i]
            pops = 0
            ai = 0
            next_w = 0
            for c_idx, chunk in enumerate(chunks):
                emit_score_chunk(i, chunk)
                # mask windows as soon as their 4 slots' exps are emitted
                nxt_slot = chunks[c_idx + 1][1] if c_idx + 1 < nS else 99
                while next_w < nwin and 4 * next_w + 3 < nxt_slot:
                    emit_mask_window(i, next_w)
                    next_w += 1
                # interleave prev unit's attv 1:1 with scores
                tgt = (c_idx + 1) * len(av) // nS
                while ai < tgt:
                    emit_attv_part(i - 1, av[ai])
                    ai += 1
                # paced projection filler
                if pops < quota and (c_idx + 1) * quota >= (pops + 1) * nS:
                    if pop_filler(i):
                        pops += 1
            if g >= 1:
                emit_text_mask(i)
            while ai < len(av):
                emit_attv_part(i - 1, av[ai])
                ai += 1
            if i >= 1:
                emit_recip_bcast(i - 1)

        # ---------- tail ----------
        nu = len(units)
        emit_norm_mult(nu - 2)
        av = attv_part_list(nu - 1)
        for ai, part in enumerate(av):
            emit_attv_part(nu - 1, part)
            if ai % 3 == 2:
                pop_filler(99)  # drain leftover f4/f3 if any
        emit_recip_bcast(nu - 1)
        emit_norm_mult(nu - 1)
        while pop_filler(99):
            pass

    nc.compile()
    return nc


def _build_mask_np(seg_starts, seg_ends):
    """True = masked. Mirrors reference._build_mask in numpy."""
    ML = 3 * T
    tril = np.tril(np.ones((T, T), dtype=bool))
    sl = np.tril(np.ones((T, T), dtype=bool), -1)
    m = np.zeros((L, L), dtype=bool)
    m[:ML, :ML] = True
    m[0:T, 0:T] = ~tril
    m[T : 2 * T, 0:T] = ~tril
    m[T : 2 * T, T : 2 * T] = ~sl
    m[T : 2 * T, 2 * T : 3 * T] = ~sl
    m[2 * T : 3 * T, 0:T] = ~tril
    m[2 * T : 3 * T, T : 2 * T] = ~tril
    m[2 * T : 3 * T, 2 * T : 3 * T] = ~sl
    m[:ML, ML:] = True
    frames = np.arange(T)[None, :, None]
    allowed = (frames >= seg_starts[:, None, :]) & (frames < seg_ends[:, None, :])
    mask = np.broadcast_to(m[None], (B, L, L)).copy()
    for row0, col_blocks in ((T, (0, 2, 3)), (2 * T, (1, 2, 3))):
        for j in col_blocks:
            c0 = ML + j * N
            mask[:, row0 : row0 + T, c0 : c0 + N] &= ~allowed
    return mask


def get_nc():
    global _NC
    if _NC is None:
        _NC = _build_program()
    return _NC


def _build_maskstk():
    r = np.arange(P)
    t1 = (r[:, None] <= r[None, :]).astype(np.float16)  # tril.T
    t2 = (r[:, None] < r[None, :]).astype(np.float16)  # strict
    stk = np.empty((P, 28, P), dtype=np.float16)
    s = 0
    for g in range(3):
        for jt in _GRP_JTS[g][: (4 if g == 0 else 12)]:
            stk[:, s, :] = t1 if _grp_diag(g, jt) == "T1" else t2
            s += 1
    assert s == 28
    return stk.reshape(P, 28 * P)


def make_in_maps(x, Wq, bq, Wk, bk, Wv, bv, Wp, bp, seg_starts, seg_ends):
    mask = _build_mask_np(np.asarray(seg_starts), np.asarray(seg_ends))
    maskstk = _build_maskstk()
    in_maps = []
    for core in range(8):
        b, g = core // 2, core % 2
        gs = slice(g * G, (g + 1) * G)
        allowT = ~mask[b].T  # [j, i]
        maskTx = np.ascontiguousarray(
            allowT[1536:1568, 512:1536].astype(np.float16)
        )
        in_maps.append(
            {
                "xT": np.ascontiguousarray(x[b].T).astype(np.float16),
                "wqT": np.ascontiguousarray(Wq[gs, :].T).astype(np.float16),
                "wkT": np.ascontiguousarray(Wk[gs, :].T).astype(np.float16),
                "wvT": np.ascontiguousarray(Wv[gs, :].T).astype(np.float16),
                "wpT": np.ascontiguousarray(Wp[:, gs].T).astype(np.float16),
                "bqP": np.ascontiguousarray(bq[gs].reshape(CT, P).T),
                "bkP": np.ascontiguousarray(bk[gs].reshape(CT, P).T),
                "maskStk": maskstk,
                "maskTxt": maskTx,
            }
        )
    return in_maps


def kernel(x, Wq, bq, Wk, bk, Wv, bv, Wp, bp, seg_starts, seg_ends, T_motion=None,
           N=None, _trace=False, **_unused):
    x = np.asarray(x, np.float32)
    args = [np.asarray(a, np.float32) for a in (Wq, bq, Wk, bk, Wv, bv, Wp, bp)]
    Wq, bq, Wk, bk, Wv, bv, Wp, bp = args
    nc = get_nc()
    in_maps = make_in_maps(x, Wq, bq, Wk, bk, Wv, bv, Wp, bp, seg_starts, seg_ends)
    res = run_bass_kernel_spmd(nc, in_maps, core_ids=list(range(8)), trace=_trace)
    parts = [np.asarray(r["out_part"], np.float32) for r in res.results]
    # v-bias folds into the output bias exactly: att rows sum to 1, so
    # y = att@(v+bv) = att@v + bv, and (y+bv)@Wp.T = y@Wp.T + bv@Wp.T
    bp_eff = bp + bv @ Wp.T
    y = np.empty((B, L, C), np.float32)
    for b in range(B):
        y[b] = parts[2 * b] + parts[2 * b + 1] + bp_eff
    if _trace:
        kernel.last_results = res
    return y


# revision 6
# speedup vs baseline: 1.2042x; 1.1695x over previous
"""Cross-conditional GPT2 sparse attention block on 8 Trainium2 NeuronCores.

Sharding: core = (batch b in 0..3) x (head-group g in 0..1, 6 heads each).

v4 schedule: one globally software-pipelined PE stream designed around the
TRN2 p-state rule (PE reaches 2.4 GHz only after ~3us of gap-free execution;
any stall drops it to 1.2 GHz):
  - projection chains (q/k/v/out-proj) are spread across the WHOLE kernel as
    PE filler, scheduled by dependency deadline, so the attention phase
    always has independent PE work between dependent score matmuls.
  - attv chunks of unit i-1 are interleaved with scores chunks of unit i.
  - exp PAIRING: pt slots are ordered [A0,B0,A1,B1,A2,B2,A3,B7-style] so two
    equal-width score chunks land in one 2-bank psum tile and ONE ACT exp
    covers both ([p, 2, W] APs) - halves ACT instruction count (ACT memory
    latency is ~185ns/instr of busy overhead).
  - PSUM: pairs 4 banks x2bufs, psY 2 banks (per-ich drain discipline),
    mixed pool 2 banks (projection chains + g2 tail chunks).
  - softmax denominator: ones-COLUMN at v_ones[...,0] puts the den row at
    PSUM partition 0 (reciprocal_approx_fast reads PSUM base partition 0
    directly); y rows at partitions 64..127 (PSUM APs cannot cross the
    64-partition boundary unless 64-aligned).
  - warmup dummy matmuls ramp the PE p-state while input DMAs land.
"""

import sys

sys.path.insert(0, "/opt/trn_rl_repo")

from collections import deque
from contextlib import ExitStack

import numpy as np

import concourse.bacc as bacc
import concourse.bass as bass
import concourse.mybir as mybir
import concourse.tile as tile
from concourse.bass_utils import run_bass_kernel_spmd

# ---- problem constants (hardcoded per spec) ----
B = 4
T = 512
N = 8
C = 768
NHEAD = 12
L = 3 * T + 4 * N  # 1568
P = 128
G = C // 2  # 384 channels per head-group
NH = 6  # heads per core
D = 64  # head dim
ET = C // P  # 6 e-tiles (contraction of x @ W)
CT = G // P  # 3 c-tiles of the group's channels
NJT = (L + P - 1) // P  # 13 j tiles (12x128 + 32)
SLOT = 544  # pt slot width per j-tile (max interval length)
I_CHUNKS = [(0, 512), (512, 512), (1024, 512), (1536, 32)]
ICH0 = (0, 512, 1024, 1536)
SCALE = 1.0 / 8.0  # 1/sqrt(64)
V0 = 64  # v rows base partition in psY (den/ones row at partition 0)
VW = V0 + D  # v_ones width 128: [0]=ones, [64:128]=v

F32 = mybir.dt.float32
F16 = mybir.dt.float16

_NC = None  # cached compiled Bass program


def _jl(jt):
    return P if jt < NJT - 1 else L - (NJT - 1) * P  # 128 or 32


def _ich_of(a):
    return 3 if a == 1536 else a // 512


# (group) -> per-jt score interval (a, ln).
# g0 = upper rows (i 0..512), jts 0..3; g1 = lower rows; g2 = torso+text rows.
def _grp_interval(g, jt):
    j0 = jt * P
    f0 = (jt % 4) * P if jt <= 11 else 0
    if g == 0:
        return (j0, 512 - j0) if jt <= 3 else None
    if g == 1:
        s = j0 if jt <= 3 else f0
        return (512 + s, 512 - s)
    s = j0 if jt <= 3 else f0
    return (1024 + s, 544 - s)


# diag mask kind per (group, jt in 0..11): 'T1' (tril.T) | 'T2' (strict)
def _grp_diag(g, jt):
    if g == 0:
        return "T1"
    if g == 1:
        return "T1" if jt <= 3 else "T2"
    return "T1" if jt <= 7 else "T2"


# pt slot order: A/B j-tile groups interleaved so exp pairs hit adjacent slots
_SLOT_JTS = {
    0: [0, 1, 2, 3],
    1: [0, 4, 1, 5, 2, 6, 3, 7, 8, 9, 10, 11, 12],
    2: [0, 4, 1, 5, 2, 6, 3, 7, 8, 9, 10, 11, 12],
}


def _unit_slots(g):
    """Per pt-slot: (jt, slot, a, main_cl, tail_cl)."""
    out = []
    for slot, jt in enumerate(_SLOT_JTS[g]):
        a, ln = _grp_interval(g, jt)
        out.append((jt, slot, a, min(ln, 512), max(0, ln - 512)))
    return out


def _build_program():
    nc = bacc.Bacc("TRN2", target_bir_lowering=False, debug=False)

    xT_d = nc.dram_tensor("xT", [C, L], F16, kind="ExternalInput")
    wq_d = nc.dram_tensor("wqT", [C, G], F16, kind="ExternalInput")
    wk_d = nc.dram_tensor("wkT", [C, G], F16, kind="ExternalInput")
    wv_d = nc.dram_tensor("wvT", [C, G], F16, kind="ExternalInput")
    wp_d = nc.dram_tensor("wpT", [G, C], F16, kind="ExternalInput")
    bq_d = nc.dram_tensor("bqP", [P, CT], F32, kind="ExternalInput")
    bk_d = nc.dram_tensor("bkP", [P, CT], F32, kind="ExternalInput")
    mstk_d = nc.dram_tensor("maskStk", [P, 28 * P], F16, kind="ExternalInput")
    maskt_d = nc.dram_tensor("maskTxt", [32, 1024], F16, kind="ExternalInput")
    out_d = nc.dram_tensor("out_part", [L, C], F16, kind="ExternalOutput")

    # mask-stack slot offset per group (g0: 4 slots, g1: 12, g2: 12)
    MOFF = {0: 0, 1: 4, 2: 16}
    units = [(g, h) for g in range(3) for h in range(NH)]

    with tile.TileContext(nc) as tc, ExitStack() as big:
        persist = big.enter_context(tc.tile_pool(name="persist", bufs=1))
        phA = big.enter_context(tc.tile_pool(name="phA", bufs=1))
        phB = big.enter_context(tc.tile_pool(name="phB", bufs=1))
        psPair = big.enter_context(tc.tile_pool(name="psPair", bufs=2, space="PSUM"))
        psYp = big.enter_context(tc.tile_pool(name="psYp", bufs=2, space="PSUM"))
        psMix = big.enter_context(tc.tile_pool(name="psMix", bufs=2, space="PSUM"))

        # persistent SBUF tensors
        qT = persist.tile([P, CT, L], F16, name="qT")
        kT = persist.tile([P, CT, L], F16, name="kT")
        v_ones = persist.tile([P, NJT, NH, VW], F16, name="v_ones")
        maskStk = persist.tile([P, 28, P], F16, name="maskStk_sb")
        maskTx = persist.tile([32, 1024], F16, name="maskTx_sb")
        yT = persist.tile([P, CT, L], F16, name="yT")
        wp_sb = persist.tile([P, CT, C], F16, name="wp_sb")
        warm = persist.tile([P, 512], F16, name="warm")

        # small memset first: PE warmup dummies depend only on it
        nc.gpsimd.memset(warm[:], 0.125)
        # ones column at free index 0 (-> den at PSUM partition 0); cols
        # 1..63 stay 1.0 but psY rows 1..63 are never read.
        nc.gpsimd.memset(v_ones[:], 1.0)

        nc.sync.dma_start(maskStk[:], mstk_d.rearrange("p (s c) -> p s c", c=P))
        nc.sync.dma_start(maskTx[:], maskt_d[:])
        nc.sync.dma_start(wp_sb[:], wp_d.rearrange("(ct p) n -> p ct n", p=P))

        # ---------- input tiles + DMA (x split by i-chunk) ----------
        xT = phA.tile([P, ET, L], F16, name="xT_sb")
        wq_sb = phA.tile([P, ET, G], F16, name="wq_sb")
        wk_sb = phA.tile([P, ET, G], F16, name="wk_sb")
        wv_sb = phA.tile([P, ET, G], F16, name="wv_sb")
        bq_sb = phA.tile([P, CT], F32, name="bq_sb")
        bk_sb = phA.tile([P, CT], F32, name="bk_sb")

        for i0, ilen in I_CHUNKS:
            for et in range(ET):
                nc.sync.dma_start(
                    xT[:, et, i0 : i0 + ilen],
                    xT_d[et * P : (et + 1) * P, i0 : i0 + ilen],
                )
        for w_sb, w_d in ((wk_sb, wk_d), (wq_sb, wq_d), (wv_sb, wv_d)):
            for et in range(ET):
                nc.sync.dma_start(w_sb[:, et, :], w_d[et * P : (et + 1) * P, :])
        nc.sync.dma_start(bq_sb[:], bq_d[:])
        nc.sync.dma_start(bk_sb[:], bk_d[:])

        # ---------- PE p-state warmup (no DMA deps) ----------
        for d in range(6):
            dt_ = psPair.tile([P, 2, 512], F32, name="ps_warm", tag="ps_pair")
            nc.tensor.matmul(
                dt_[:, d % 2, :],
                warm[:, 0:P],
                warm[:, :],
                start=True,
                stop=True,
                skip_group_check=True,
            )

        # ---------- projection chain emitters ----------
        def emit_qk_chain(dst, w_sb, b_sb, ct, ich):
            i0, ilen = I_CHUNKS[ich]
            ps = psMix.tile([P, 512], F32, name="ps_p", tag="ps_mix")
            for et in range(ET):
                nc.tensor.matmul(
                    ps[:, :ilen],
                    w_sb[:, et, ct * P : (ct + 1) * P],
                    xT[:, et, i0 : i0 + ilen],
                    start=(et == 0),
                    stop=(et == ET - 1),
                    skip_group_check=True,
                )
            nc.vector.tensor_scalar(
                dst[:, ct, i0 : i0 + ilen],
                ps[:, :ilen],
                b_sb[:, ct : ct + 1],
                None,
                mybir.AluOpType.add,
            )

        def emit_v_chain(it):
            il = _jl(it)
            ps = psMix.tile([P, 512], F32, name="ps_pv", tag="ps_mix")
            for et in range(ET):
                nc.tensor.matmul(
                    ps[:il, :G],
                    xT[:, et, it * P : it * P + il],
                    wv_sb[:, et, :],
                    start=(et == 0),
                    stop=(et == ET - 1),
                    skip_group_check=True,
                )
            nc.vector.tensor_copy(
                v_ones[:il, it, :, V0 : V0 + D],
                ps[:il, :G].rearrange("p (h d) -> p h d", h=NH),
            )

        def emit_outproj_chain(it, nch, tail=False):
            il = _jl(it)
            ps_o = psMix.tile([P, 512], F32, name="ps_po", tag="ps_mix")
            for kt in range(CT):
                nc.tensor.matmul(
                    ps_o[:il, :G],
                    yT[:, kt, it * P : it * P + il],
                    wp_sb[:, kt, nch * G : (nch + 1) * G],
                    start=(kt == 0),
                    stop=(kt == CT - 1),
                    skip_group_check=True,
                )
            o_sb = phB.tile([P, G], F16, name="o_sb", tag="o_sb", bufs=6)
            nc.vector.tensor_copy(o_sb[:il, :], ps_o[:il, :G])
            r0 = it * P
            c0 = nch * G
            if tail and il == P:
                # split across queues + issue engines to shorten final drain
                e2 = nc.scalar if (it + nch) % 2 == 0 else nc.sync
                nc.sync.dma_start(out_d[r0 : r0 + 64, c0 : c0 + G], o_sb[0:64, :])
                e2.dma_start(out_d[r0 + 64 : r0 + il, c0 : c0 + G], o_sb[64:il, :])
            else:
                nc.sync.dma_start(out_d[r0 : r0 + il, c0 : c0 + G], o_sb[:il, :])

        # ---------- attention emitters ----------
        urec = [dict() for _ in units]

        def kq(i, jt, ca, cl):
            g, h = units[i]
            pof = D * (h % 2)
            ct = h // 2
            jl = _jl(jt)
            return (
                kT[pof : pof + D, ct, jt * P : jt * P + jl],
                qT[pof : pof + D, ct, ca : ca + cl],
            )

        def emit_pair(i, s1, s2):
            # two equal-ish chunks -> one 2-bank psum tile -> ONE exp
            jt1, slot1, a1, cl1, _ = s1
            jt2, slot2, a2, cl2, _ = s2
            pt = urec[i]["pt"]
            tile_ = psPair.tile([P, 2, 512], F32, name="ps_s2", tag="ps_pair")
            for idx, (jt, ca, cl) in enumerate(((jt1, a1, cl1), (jt2, a2, cl2))):
                k_ap, q_ap = kq(i, jt, ca, cl)
                nc.tensor.matmul(
                    tile_[: _jl(jt), idx, :cl], k_ap, q_ap,
                    start=True, stop=True, skip_group_check=True,
                )
            W = max(cl1, cl2)
            nc.scalar.activation(
                pt[:P, slot1 : slot1 + 2, 0:W],
                tile_[:P, :, 0:W],
                mybir.ActivationFunctionType.Exp,
                bias=0.0,
                scale=SCALE,
            )

        def emit_single12(i, s):
            jt, slot, a, mcl, tcl = s
            pt = urec[i]["pt"]
            jl = _jl(jt)  # 32
            tile_ = psPair.tile([P, 2, 512], F32, name="ps_s1", tag="ps_pair")
            k_ap, q_ap = kq(i, jt, a, mcl)
            nc.tensor.matmul(
                tile_[:jl, 0, :mcl], k_ap, q_ap,
                start=True, stop=True, skip_group_check=True,
            )
            if tcl:
                k_ap, q_ap = kq(i, jt, a + 512, tcl)
                nc.tensor.matmul(
                    tile_[:jl, 1, :tcl], k_ap, q_ap,
                    start=True, stop=True, skip_group_check=True,
                )
            flat = tile_[:jl].rearrange("p a b -> p (a b)")
            nc.scalar.activation(
                pt[:jl, slot, 0 : mcl + tcl],
                flat[:, 0 : mcl + tcl],
                mybir.ActivationFunctionType.Exp,
                bias=0.0,
                scale=SCALE,
            )

        def emit_tailpair(i, s1, s2):
            # the 32-col score tails of two adjacent 544-wide slots
            pt = urec[i]["pt"]
            mix = psMix.tile([P, 512], F32, name="ps_st", tag="ps_mix")
            for idx, (jt, slot, a, mcl, tcl) in enumerate((s1, s2)):
                k_ap, q_ap = kq(i, jt, a + 512, tcl)
                nc.tensor.matmul(
                    mix[: _jl(jt), idx * 32 : idx * 32 + 32], k_ap, q_ap,
                    start=True, stop=True, skip_group_check=True,
                )
            nc.scalar.activation(
                pt[:P, s1[1] : s1[1] + 2, 512:544],
                mix[:P, 0:64].rearrange("p (a b) -> p a b", b=32),
                mybir.ActivationFunctionType.Exp,
                bias=0.0,
                scale=SCALE,
            )

        def emit_tail8(i, s):
            jt, slot, a, mcl, tcl = s
            pt = urec[i]["pt"]
            mix = psMix.tile([P, 512], F32, name="ps_st8", tag="ps_mix")
            k_ap, q_ap = kq(i, jt, a + 512, tcl)
            nc.tensor.matmul(
                mix[: _jl(jt), 0:32], k_ap, q_ap,
                start=True, stop=True, skip_group_check=True,
            )
            nc.scalar.activation(
                pt[:P, slot, 512:544],
                mix[:P, 0:32],
                mybir.ActivationFunctionType.Exp,
                bias=0.0,
                scale=SCALE,
            )

        def emit_mask_window(i, w):
            g, h = units[i]
            pt = urec[i]["pt"]
            s0 = 4 * w
            nc.vector.tensor_tensor(
                pt[:, s0 : s0 + 4, 0:P],
                pt[:, s0 : s0 + 4, 0:P],
                maskStk[:, MOFF[g] + s0 : MOFF[g] + s0 + 4, :],
                mybir.AluOpType.mult,
            )

        def emit_text_mask(i):
            g, h = units[i]
            pt = urec[i]["pt"]
            m0 = 0 if g == 1 else 512
            nc.vector.tensor_tensor(
                pt[0:32, 12, 0:512],
                pt[0:32, 12, 0:512],
                maskTx[0:32, m0 : m0 + 512],
                mybir.AluOpType.mult,
            )

        def attv_items(i):
            """Flat list: ("part", ich, jt, slot, pl, poff, off, first, stop)
            and ("drain", ich, is_final) items, ich groups in drain order
            (g2: ich3 first - its psY bank frees mid-block)."""
            g, h = units[i]
            groups = {}
            for jt, slot, ca, cl, soff in urec[i]["chunks"]:
                subs = [(ca, cl, soff)]
                if ca < 1536 < ca + cl:
                    subs = [
                        (ca, 1536 - ca, soff),
                        (1536, ca + cl - 1536, soff + 1536 - ca),
                    ]
                for pa, pl, poff in subs:
                    ich = _ich_of(pa)
                    groups.setdefault(ich, []).append(
                        (jt, slot, pa, pl, poff)
                    )
            ich_order = sorted(groups, key=lambda c: len(groups[c]) * 1000 + c)
            if units[i][0] == 2:
                ich_order = [3, 2]
            items = []
            for gi, ich in enumerate(ich_order):
                parts = groups[ich]
                for pi, (jt, slot, pa, pl, poff) in enumerate(parts):
                    items.append(
                        ("part", ich, jt, slot, pa, pl, poff,
                         pi == 0, pi == len(parts) - 1)
                    )
                items.append(("drain", ich, gi == len(ich_order) - 1))
            return items

        def emit_attv_item(i, item):
            g, h = units[i]
            if item[0] == "part":
                _, ich, jt, slot, pa, pl, poff, first, stop = item
                jl = _jl(jt)
                ysd = urec[i].setdefault("ps_y", {})
                if ich not in ysd:
                    ysd[ich] = psYp.tile([VW, 512], F32, name="ps_y", tag="ps_y")
                off = pa - ICH0[ich]
                nc.tensor.matmul(
                    ysd[ich][:, off : off + pl],
                    v_ones[:jl, jt, h, :],
                    urec[i]["pt"][:jl, slot, poff : poff + pl],
                    start=first,
                    stop=stop,
                    skip_group_check=True,
                )
            else:
                _, ich, is_final = item
                psy = urec[i]["ps_y"][ich]
                ilen = I_CHUNKS[ich][1]
                rc = phB.tile([1, 512], F32, name="rc", tag="rc", bufs=4)
                nc.vector.reciprocal_approx_fast(
                    out=rc[0:1, :ilen], in_=psy[0:1, :ilen]
                )
                rc_bc = phB.tile([D, 512], F32, name="rc_bc", tag="rc_bc", bufs=4)
                nc.gpsimd.partition_broadcast(rc_bc[:, :ilen], rc[0:1, :ilen])
                if is_final:
                    urec[i]["pending"] = (ich, psy, rc_bc)
                else:
                    emit_norm(i, ich, psy, rc_bc)

        def emit_norm(i, ich, psy, rc_bc):
            g, h = units[i]
            pof = D * (h % 2)
            ct = h // 2
            i0, ilen = I_CHUNKS[ich]
            nc.vector.tensor_tensor(
                yT[pof : pof + D, ct, i0 : i0 + ilen],
                psy[V0 : V0 + D, :ilen],
                rc_bc[:, :ilen],
                mybir.AluOpType.mult,
            )

        def emit_pending_norm(i):
            if "pending" in urec[i]:
                ich, psy, rc_bc = urec[i].pop("pending")
                emit_norm(i, ich, psy, rc_bc)

        # ---------- filler segments ----------
        f1 = deque()
        for ct in range(CT):
            if ct == 0:
                f1.append(lambda: emit_qk_chain(qT, wq_sb, bq_sb, 0, 1))
                for ich in (1, 2, 3):
                    f1.append(
                        lambda ich=ich: emit_qk_chain(kT, wk_sb, bk_sb, 0, ich)
                    )
                for it in range(4, NJT):
                    f1.append(lambda it=it: emit_v_chain(it))
            else:
                f1.append(lambda ct=ct: emit_qk_chain(qT, wq_sb, bq_sb, ct, 1))
                for ich in (1, 2, 3):
                    f1.append(
                        lambda ct=ct, ich=ich: emit_qk_chain(
                            kT, wk_sb, bk_sb, ct, ich
                        )
                    )
        f2 = deque()
        for ich in (2, 3):
            for ct in range(CT):
                f2.append(
                    lambda ct=ct, ich=ich: emit_qk_chain(qT, wq_sb, bq_sb, ct, ich)
                )
        f3 = deque(
            (lambda it=it, nch=nch: emit_outproj_chain(it, nch))
            for it in range(0, 4)
            for nch in range(2)
        )
        f4 = deque(
            (lambda it=it, nch=nch: emit_outproj_chain(it, nch))
            for it in range(4, 8)
            for nch in range(2)
        )
        segments = [(0, f1), (6, f2), (7, f3), (13, f4)]

        def pop_filler(block):
            for rel, dq in segments:
                if rel <= block and dq:
                    dq.popleft()()
                    return True
            return False

        QUOTA = [4, 4, 4, 3, 3, 3, 2, 2, 2, 2, 2, 2, 2, 2, 2, 2, 2, 2]

        # ---------- upfront: phase A for g0's needs ----------
        for ct in range(CT):
            emit_qk_chain(kT, wk_sb, bk_sb, ct, 0)
            emit_qk_chain(qT, wq_sb, bq_sb, ct, 0)
        for it in range(4):
            emit_v_chain(it)

        # ---------- main software-pipelined loop ----------
        for i, (g, h) in enumerate(units):
            if i == 6:
                while f1:
                    f1.popleft()()
            if i == 12:
                while f2:
                    f2.popleft()()
            if i >= 2:
                emit_pending_norm(i - 2)
            urec[i]["pt"] = phB.tile(
                [P, NJT, SLOT], F16, name="pt", tag="pt", bufs=3
            )
            slots = _unit_slots(g)
            chunks = []
            for jt, slot, a, mcl, tcl in slots:
                chunks.append((jt, slot, a, mcl, 0))
                if tcl:
                    chunks.append((jt, slot, a + 512, tcl, 512))
            urec[i]["chunks"] = chunks

            # exp-units: (fn, main_slot_done)
            eus = []
            if g == 0:
                eus.append((lambda s=slots: emit_pair(i, s[0], s[1]), 1))
                eus.append((lambda s=slots: emit_pair(i, s[2], s[3]), 3))
            else:
                for k in range(6):
                    eus.append(
                        (lambda s=slots, k=k: emit_pair(i, s[2 * k], s[2 * k + 1]),
                         2 * k + 1)
                    )
                eus.append((lambda s=slots: emit_single12(i, s[12]), 12))
                if g == 2:
                    eus.append((lambda s=slots: emit_tailpair(i, s[0], s[1]), -1))
                    eus.append((lambda s=slots: emit_tail8(i, s[8]), -1))

            av = attv_items(i - 1) if i >= 1 else []
            nE = len(eus)
            nwin = 1 if g == 0 else 3
            quota = QUOTA[i]
            pops = 0
            ai = 0
            next_w = 0
            for e_idx, (fn, sdone) in enumerate(eus):
                fn()
                while next_w < nwin and 4 * next_w + 3 <= sdone:
                    emit_mask_window(i, next_w)
                    next_w += 1
                if sdone == 12 and g >= 1:
                    emit_text_mask(i)
                tgt = (e_idx + 1) * len(av) // nE
                while ai < tgt:
                    emit_attv_item(i - 1, av[ai])
                    ai += 1
                if pops < quota and (e_idx + 1) * quota >= (pops + 1) * nE:
                    if pop_filler(i):
                        pops += 1
            while ai < len(av):
                emit_attv_item(i - 1, av[ai])
                ai += 1

        # ---------- tail ----------
        nu = len(units)
        emit_pending_norm(nu - 2)
        av = attv_items(nu - 1)
        for item in av:
            emit_attv_item(nu - 1, item)
            if item[0] == "drain" and item[1] == 3:
                # it12 out-proj needs only the ich3 norms (text rows), which
                # just completed - run it while the ich2 attv still streams
                for nch in range(2):
                    emit_outproj_chain(12, nch, tail=True)
        emit_pending_norm(nu - 1)
        for it in range(8, 12):
            for nch in range(2):
                emit_outproj_chain(it, nch, tail=True)

    nc.compile()
    return nc


def _build_mask_np(seg_starts, seg_ends):
    """True = masked. Mirrors reference._build_mask in numpy."""
    ML = 3 * T
    tril = np.tril(np.ones((T, T), dtype=bool))
    sl = np.tril(np.ones((T, T), dtype=bool), -1)
    m = np.zeros((L, L), dtype=bool)
    m[:ML, :ML] = True
    m[0:T, 0:T] = ~tril
    m[T : 2 * T, 0:T] = ~tril
    m[T : 2 * T, T : 2 * T] = ~sl
    m[T : 2 * T, 2 * T : 3 * T] = ~sl
    m[2 * T : 3 * T, 0:T] = ~tril
    m[2 * T : 3 * T, T : 2 * T] = ~tril
    m[2 * T : 3 * T, 2 * T : 3 * T] = ~sl
    m[:ML, ML:] = True
    frames = np.arange(T)[None, :, None]
    allowed = (frames >= seg_starts[:, None, :]) & (frames < seg_ends[:, None, :])
    mask = np.broadcast_to(m[None], (B, L, L)).copy()
    for row0, col_blocks in ((T, (0, 2, 3)), (2 * T, (1, 2, 3))):
        for j in col_blocks:
            c0 = ML + j * N
            mask[:, row0 : row0 + T, c0 : c0 + N] &= ~allowed
    return mask


def get_nc():
    global _NC
    if _NC is None:
        _NC = _build_program()
    return _NC


def _build_maskstk():
    r = np.arange(P)
    t1 = (r[:, None] <= r[None, :]).astype(np.float16)  # tril.T
    t2 = (r[:, None] < r[None, :]).astype(np.float16)  # strict
    stk = np.empty((P, 28, P), dtype=np.float16)
    s = 0
    for g in range(3):
        ndiag = 4 if g == 0 else 12
        for jt in _SLOT_JTS[g][:ndiag]:
            stk[:, s, :] = t1 if _grp_diag(g, jt) == "T1" else t2
            s += 1
    assert s == 28
    return stk.reshape(P, 28 * P)


def make_in_maps(x, Wq, bq, Wk, bk, Wv, bv, Wp, bp, seg_starts, seg_ends):
    mask = _build_mask_np(np.asarray(seg_starts), np.asarray(seg_ends))
    maskstk = _build_maskstk()
    in_maps = []
    for core in range(8):
        b, g = core // 2, core % 2
        gs = slice(g * G, (g + 1) * G)
        allowT = ~mask[b].T  # [j, i]
        maskTx = np.ascontiguousarray(
            allowT[1536:1568, 512:1536].astype(np.float16)
        )
        in_maps.append(
            {
                "xT": np.ascontiguousarray(x[b].T).astype(np.float16),
                "wqT": np.ascontiguousarray(Wq[gs, :].T).astype(np.float16),
                "wkT": np.ascontiguousarray(Wk[gs, :].T).astype(np.float16),
                "wvT": np.ascontiguousarray(Wv[gs, :].T).astype(np.float16),
                "wpT": np.ascontiguousarray(Wp[:, gs].T).astype(np.float16),
                "bqP": np.ascontiguousarray(bq[gs].reshape(CT, P).T),
                "bkP": np.ascontiguousarray(bk[gs].reshape(CT, P).T),
                "maskStk": maskstk,
                "maskTxt": maskTx,
            }
        )
    return in_maps


def kernel(x, Wq, bq, Wk, bk, Wv, bv, Wp, bp, seg_starts, seg_ends, T_motion=None,
           N=None, _trace=False, **_unused):
    x = np.asarray(x, np.float32)
    args = [np.asarray(a, np.float32) for a in (Wq, bq, Wk, bk, Wv, bv, Wp, bp)]
    Wq, bq, Wk, bk, Wv, bv, Wp, bp = args
    nc = get_nc()
    in_maps = make_in_maps(x, Wq, bq, Wk, bk, Wv, bv, Wp, bp, seg_starts, seg_ends)
    res = run_bass_kernel_spmd(nc, in_maps, core_ids=list(range(8)), trace=_trace)
    parts = [np.asarray(r["out_part"], np.float32) for r in res.results]
    # v-bias folds into the output bias exactly: att rows sum to 1, so
    # y = att@(v+bv) = att@v + bv, and (y+bv)@Wp.T = y@Wp.T + bv@Wp.T
    bp_eff = bp + bv @ Wp.T
    y = np.empty((B, L, C), np.float32)
    for b in range(B):
        y[b] = parts[2 * b] + parts[2 * b + 1] + bp_eff
    if _trace:
        kernel.last_results = res
    return y


# revision 7
# speedup vs baseline: 1.3859x; 1.1509x over previous
"""Cross-conditional GPT2 sparse attention block on 8 Trainium2 NeuronCores.

Sharding: core = (batch b in 0..3) x (head-group g in 0..1, 6 heads each).

v4 schedule: one globally software-pipelined PE stream designed around the
TRN2 p-state rule (PE reaches 2.4 GHz only after ~3us of gap-free execution;
any stall drops it to 1.2 GHz):
  - projection chains (q/k/v/out-proj) are spread across the WHOLE kernel as
    PE filler, scheduled by dependency deadline, so the attention phase
    always has independent PE work between dependent score matmuls.
  - attv chunks of unit i-1 are interleaved with scores chunks of unit i.
  - exp PAIRING: pt slots are ordered [A0,B0,A1,B1,A2,B2,A3,B7-style] so two
    equal-width score chunks land in one 2-bank psum tile and ONE ACT exp
    covers both ([p, 2, W] APs) - halves ACT instruction count (ACT memory
    latency is ~185ns/instr of busy overhead).
  - PSUM: pairs 4 banks x2bufs, psY 2 banks (per-ich drain discipline),
    mixed pool 2 banks (projection chains + g2 tail chunks).
  - softmax denominator: ones-COLUMN at v_ones[...,0] puts the den row at
    PSUM partition 0 (reciprocal_approx_fast reads PSUM base partition 0
    directly); y rows at partitions 64..127 (PSUM APs cannot cross the
    64-partition boundary unless 64-aligned).
  - warmup dummy matmuls ramp the PE p-state while input DMAs land.
"""

import sys

sys.path.insert(0, "/opt/trn_rl_repo")

from collections import deque
from contextlib import ExitStack

import numpy as np

import concourse.bacc as bacc
import concourse.bass as bass
import concourse.mybir as mybir
import concourse.tile as tile
from concourse.bass_utils import run_bass_kernel_spmd

# ---- problem constants (hardcoded per spec) ----
B = 4
T = 512
N = 8
C = 768
NHEAD = 12
L = 3 * T + 4 * N  # 1568
P = 128
G = C // 2  # 384 channels per head-group
NH = 6  # heads per core
D = 64  # head dim
ET = C // P  # 6 e-tiles (contraction of x @ W)
CT = G // P  # 3 c-tiles of the group's channels
NJT = (L + P - 1) // P  # 13 j tiles (12x128 + 32)
SLOT = 544  # pt slot width per j-tile (max interval length)
I_CHUNKS = [(0, 512), (512, 512), (1024, 512), (1536, 32)]
ICH0 = (0, 512, 1024, 1536)
SCALE = 1.0 / 8.0  # 1/sqrt(64)
V0 = 64  # v rows base partition in psY (den/ones row at partition 0)
VW = V0 + D  # v_ones width 128: [0]=ones, [64:128]=v

F32 = mybir.dt.float32
F16 = mybir.dt.float16

_NC = None  # cached compiled Bass program


def _jl(jt):
    return P if jt < NJT - 1 else L - (NJT - 1) * P  # 128 or 32


def _ich_of(a):
    return 3 if a == 1536 else a // 512


# (group) -> per-jt score interval (a, ln).
# g0 = upper rows (i 0..512), jts 0..3; g1 = lower rows; g2 = torso+text rows.
def _grp_interval(g, jt):
    j0 = jt * P
    f0 = (jt % 4) * P if jt <= 11 else 0
    if g == 0:
        return (j0, 512 - j0) if jt <= 3 else None
    if g == 1:
        s = j0 if jt <= 3 else f0
        return (512 + s, 512 - s)
    s = j0 if jt <= 3 else f0
    return (1024 + s, 544 - s)


# diag mask kind per (group, jt in 0..11): 'T1' (tril.T) | 'T2' (strict)
def _grp_diag(g, jt):
    if g == 0:
        return "T1"
    if g == 1:
        return "T1" if jt <= 3 else "T2"
    return "T1" if jt <= 7 else "T2"


# pt slot order: A/B j-tile groups interleaved so exp pairs hit adjacent slots
_SLOT_JTS = {
    0: [0, 1, 2, 3],
    1: [0, 4, 1, 5, 2, 6, 3, 7, 8, 9, 10, 11, 12],
    2: [0, 4, 1, 5, 2, 6, 3, 7, 8, 9, 10, 11, 12],
}


def _unit_slots(g):
    """Per pt-slot: (jt, slot, a, main_cl, tail_cl)."""
    out = []
    for slot, jt in enumerate(_SLOT_JTS[g]):
        a, ln = _grp_interval(g, jt)
        out.append((jt, slot, a, min(ln, 512), max(0, ln - 512)))
    return out


def _build_program():
    nc = bacc.Bacc("TRN2", target_bir_lowering=False, debug=False)

    xT_d = nc.dram_tensor("xT", [C, L], F16, kind="ExternalInput")
    wq_d = nc.dram_tensor("wqT", [C, G], F16, kind="ExternalInput")
    wk_d = nc.dram_tensor("wkT", [C, G], F16, kind="ExternalInput")
    wv_d = nc.dram_tensor("wvT", [C, G], F16, kind="ExternalInput")
    wp_d = nc.dram_tensor("wpT", [G, C], F16, kind="ExternalInput")
    bq_d = nc.dram_tensor("bqP", [P, CT], F32, kind="ExternalInput")
    bk_d = nc.dram_tensor("bkP", [P, CT], F32, kind="ExternalInput")
    mstk_d = nc.dram_tensor("maskStk", [P, 28 * P], F16, kind="ExternalInput")
    maskt_d = nc.dram_tensor("maskTxt", [32, 1024], F16, kind="ExternalInput")
    out_d = nc.dram_tensor("out_part", [L, C], F16, kind="ExternalOutput")

    # mask-stack slot offset per group (g0: 4 slots, g1: 12, g2: 12)
    MOFF = {0: 0, 1: 4, 2: 16}
    units = [(g, h) for g in range(3) for h in range(NH)]

    with tile.TileContext(nc) as tc, ExitStack() as big:
        persist = big.enter_context(tc.tile_pool(name="persist", bufs=1))
        phA = big.enter_context(tc.tile_pool(name="phA", bufs=1))
        phB = big.enter_context(tc.tile_pool(name="phB", bufs=1))
        psPair = big.enter_context(tc.tile_pool(name="psPair", bufs=2, space="PSUM"))
        psYp = big.enter_context(tc.tile_pool(name="psYp", bufs=2, space="PSUM"))
        psMix = big.enter_context(tc.tile_pool(name="psMix", bufs=2, space="PSUM"))

        # persistent SBUF tensors
        qT = persist.tile([P, CT, L], F16, name="qT")
        kT = persist.tile([P, CT, L], F16, name="kT")
        v_ones = persist.tile([P, NJT, NH, VW], F16, name="v_ones")
        maskStk = persist.tile([P, 28, P], F16, name="maskStk_sb")
        maskTx = persist.tile([32, 1024], F16, name="maskTx_sb")
        yT = persist.tile([P, CT, L], F16, name="yT")
        wp_sb = persist.tile([P, CT, C], F16, name="wp_sb")
        warm = persist.tile([P, 512], F16, name="warm")

        # small memset first: PE warmup dummies depend only on it
        nc.gpsimd.memset(warm[:], 0.125)
        # ones column at free index 0 (-> den at PSUM partition 0); cols
        # 1..63 stay 1.0 but psY rows 1..63 are never read.
        nc.gpsimd.memset(v_ones[:], 1.0)

        # ---------- input tiles + DMA, critical-path order ----------
        # First the 12 tiles the very first k/q chains need (wk, x-ich0),
        # then wq/biases/wv, then the big non-critical loads (maskStk, wp)
        # split into chunks so no queue is hogged for 30us.
        xT = phA.tile([P, ET, L], F16, name="xT_sb")
        wq_sb = phA.tile([P, ET, G], F16, name="wq_sb")
        wk_sb = phA.tile([P, ET, G], F16, name="wk_sb")
        wv_sb = phA.tile([P, ET, G], F16, name="wv_sb")
        bq_sb = phA.tile([P, CT], F32, name="bq_sb")
        bk_sb = phA.tile([P, CT], F32, name="bk_sb")

        for et in range(ET):
            nc.sync.dma_start(wk_sb[:, et, :], wk_d[et * P : (et + 1) * P, :])
            nc.sync.dma_start(xT[:, et, 0:512], xT_d[et * P : (et + 1) * P, 0:512])
        nc.sync.dma_start(bk_sb[:], bk_d[:])
        nc.sync.dma_start(bq_sb[:], bq_d[:])
        for et in range(ET):
            nc.sync.dma_start(wq_sb[:, et, :], wq_d[et * P : (et + 1) * P, :])
        for et in range(ET):
            nc.sync.dma_start(wv_sb[:, et, :], wv_d[et * P : (et + 1) * P, :])
        mstk_v = mstk_d.rearrange("p (s c) -> p s c", c=P)
        for q in range(4):
            nc.sync.dma_start(maskStk[:, 7 * q : 7 * (q + 1), :],
                              mstk_v[:, 7 * q : 7 * (q + 1), :])
        nc.sync.dma_start(maskTx[:], maskt_d[:])
        for i0, ilen in I_CHUNKS[1:]:
            for et in range(ET):
                nc.sync.dma_start(
                    xT[:, et, i0 : i0 + ilen],
                    xT_d[et * P : (et + 1) * P, i0 : i0 + ilen],
                )
        wp_v = wp_d.rearrange("(ct p) n -> p ct n", p=P)
        for ct in range(CT):
            nc.sync.dma_start(wp_sb[:, ct, :], wp_v[:, ct, :])

        # ---------- PE p-state warmup (no DMA deps) ----------
        for d in range(10):
            dt_ = psPair.tile([P, 2, 512], F32, name="ps_warm", tag="ps_pair")
            nc.tensor.matmul(
                dt_[:, d % 2, :],
                warm[:, 0:P],
                warm[:, :],
                start=True,
                stop=True,
                skip_group_check=True,
            )

        # ---------- projection chain emitters ----------
        def emit_qk_chain(dst, w_sb, b_sb, ct, ich):
            i0, ilen = I_CHUNKS[ich]
            ps = psMix.tile([P, 512], F32, name="ps_p", tag="ps_mix")
            for et in range(ET):
                nc.tensor.matmul(
                    ps[:, :ilen],
                    w_sb[:, et, ct * P : (ct + 1) * P],
                    xT[:, et, i0 : i0 + ilen],
                    start=(et == 0),
                    stop=(et == ET - 1),
                    skip_group_check=True,
                )
            nc.vector.tensor_scalar(
                dst[:, ct, i0 : i0 + ilen],
                ps[:, :ilen],
                b_sb[:, ct : ct + 1],
                None,
                mybir.AluOpType.add,
            )

        def emit_v_chain(it):
            il = _jl(it)
            ps = psMix.tile([P, 512], F32, name="ps_pv", tag="ps_mix")
            for et in range(ET):
                nc.tensor.matmul(
                    ps[:il, :G],
                    xT[:, et, it * P : it * P + il],
                    wv_sb[:, et, :],
                    start=(et == 0),
                    stop=(et == ET - 1),
                    skip_group_check=True,
                )
            nc.vector.tensor_copy(
                v_ones[:il, it, :, V0 : V0 + D],
                ps[:il, :G].rearrange("p (h d) -> p h d", h=NH),
            )

        def emit_outproj_chain(it, nch, tail=False):
            il = _jl(it)
            ps_o = psMix.tile([P, 512], F32, name="ps_po", tag="ps_mix")
            for kt in range(CT):
                nc.tensor.matmul(
                    ps_o[:il, :G],
                    yT[:, kt, it * P : it * P + il],
                    wp_sb[:, kt, nch * G : (nch + 1) * G],
                    start=(kt == 0),
                    stop=(kt == CT - 1),
                    skip_group_check=True,
                )
            o_sb = phB.tile([P, G], F16, name="o_sb", tag="o_sb", bufs=6)
            nc.vector.tensor_copy(o_sb[:il, :], ps_o[:il, :G])
            r0 = it * P
            c0 = nch * G
            if tail and il == P:
                # split across queues + issue engines to shorten final drain
                e2 = nc.scalar if (it + nch) % 2 == 0 else nc.sync
                nc.sync.dma_start(out_d[r0 : r0 + 64, c0 : c0 + G], o_sb[0:64, :])
                e2.dma_start(out_d[r0 + 64 : r0 + il, c0 : c0 + G], o_sb[64:il, :])
            else:
                nc.sync.dma_start(out_d[r0 : r0 + il, c0 : c0 + G], o_sb[:il, :])

        # ---------- attention emitters ----------
        urec = [dict() for _ in units]

        def kq(i, jt, ca, cl):
            g, h = units[i]
            pof = D * (h % 2)
            ct = h // 2
            jl = _jl(jt)
            return (
                kT[pof : pof + D, ct, jt * P : jt * P + jl],
                qT[pof : pof + D, ct, ca : ca + cl],
            )

        def emit_pair(i, s1, s2):
            # two equal-ish chunks -> one 2-bank psum tile -> ONE exp
            jt1, slot1, a1, cl1, _ = s1
            jt2, slot2, a2, cl2, _ = s2
            pt = urec[i]["pt"]
            tile_ = psPair.tile([P, 2, 512], F32, name="ps_s2", tag="ps_pair")
            for idx, (jt, ca, cl) in enumerate(((jt1, a1, cl1), (jt2, a2, cl2))):
                k_ap, q_ap = kq(i, jt, ca, cl)
                nc.tensor.matmul(
                    tile_[: _jl(jt), idx, :cl], k_ap, q_ap,
                    start=True, stop=True, skip_group_check=True,
                )
            W = max(cl1, cl2)
            nc.scalar.activation(
                pt[:P, slot1 : slot1 + 2, 0:W],
                tile_[:P, :, 0:W],
                mybir.ActivationFunctionType.Exp,
                bias=0.0,
                scale=SCALE,
            )

        def emit_single12(i, s):
            jt, slot, a, mcl, tcl = s
            pt = urec[i]["pt"]
            jl = _jl(jt)  # 32
            tile_ = psPair.tile([P, 2, 512], F32, name="ps_s1", tag="ps_pair")
            k_ap, q_ap = kq(i, jt, a, mcl)
            nc.tensor.matmul(
                tile_[:jl, 0, :mcl], k_ap, q_ap,
                start=True, stop=True, skip_group_check=True,
            )
            if tcl:
                k_ap, q_ap = kq(i, jt, a + 512, tcl)
                nc.tensor.matmul(
                    tile_[:jl, 1, :tcl], k_ap, q_ap,
                    start=True, stop=True, skip_group_check=True,
                )
            flat = tile_[:jl].rearrange("p a b -> p (a b)")
            nc.scalar.activation(
                pt[:jl, slot, 0 : mcl + tcl],
                flat[:, 0 : mcl + tcl],
                mybir.ActivationFunctionType.Exp,
                bias=0.0,
                scale=SCALE,
            )

        def emit_tailpair(i, s1, s2):
            # the 32-col score tails of two adjacent 544-wide slots
            pt = urec[i]["pt"]
            mix = psMix.tile([P, 512], F32, name="ps_st", tag="ps_mix")
            for idx, (jt, slot, a, mcl, tcl) in enumerate((s1, s2)):
                k_ap, q_ap = kq(i, jt, a + 512, tcl)
                nc.tensor.matmul(
                    mix[: _jl(jt), idx * 32 : idx * 32 + 32], k_ap, q_ap,
                    start=True, stop=True, skip_group_check=True,
                )
            nc.scalar.activation(
                pt[:P, s1[1] : s1[1] + 2, 512:544],
                mix[:P, 0:64].rearrange("p (a b) -> p a b", b=32),
                mybir.ActivationFunctionType.Exp,
                bias=0.0,
                scale=SCALE,
            )

        def emit_tail8(i, s):
            jt, slot, a, mcl, tcl = s
            pt = urec[i]["pt"]
            mix = psMix.tile([P, 512], F32, name="ps_st8", tag="ps_mix")
            k_ap, q_ap = kq(i, jt, a + 512, tcl)
            nc.tensor.matmul(
                mix[: _jl(jt), 0:32], k_ap, q_ap,
                start=True, stop=True, skip_group_check=True,
            )
            nc.scalar.activation(
                pt[:P, slot, 512:544],
                mix[:P, 0:32],
                mybir.ActivationFunctionType.Exp,
                bias=0.0,
                scale=SCALE,
            )

        def emit_mask_window(i, w):
            g, h = units[i]
            pt = urec[i]["pt"]
            s0 = 4 * w
            nc.vector.tensor_tensor(
                pt[:, s0 : s0 + 4, 0:P],
                pt[:, s0 : s0 + 4, 0:P],
                maskStk[:, MOFF[g] + s0 : MOFF[g] + s0 + 4, :],
                mybir.AluOpType.mult,
            )

        def emit_text_mask(i):
            g, h = units[i]
            pt = urec[i]["pt"]
            m0 = 0 if g == 1 else 512
            nc.vector.tensor_tensor(
                pt[0:32, 12, 0:512],
                pt[0:32, 12, 0:512],
                maskTx[0:32, m0 : m0 + 512],
                mybir.AluOpType.mult,
            )

        def attv_items(i):
            """Flat list: ("part", ich, jt, slot, pl, poff, off, first, stop)
            and ("drain", ich, is_final) items, ich groups in drain order
            (g2: ich3 first - its psY bank frees mid-block)."""
            g, h = units[i]
            groups = {}
            for jt, slot, ca, cl, soff in urec[i]["chunks"]:
                subs = [(ca, cl, soff)]
                if ca < 1536 < ca + cl:
                    subs = [
                        (ca, 1536 - ca, soff),
                        (1536, ca + cl - 1536, soff + 1536 - ca),
                    ]
                for pa, pl, poff in subs:
                    ich = _ich_of(pa)
                    groups.setdefault(ich, []).append(
                        (jt, slot, pa, pl, poff)
                    )
            ich_order = sorted(groups, key=lambda c: len(groups[c]) * 1000 + c)
            if units[i][0] == 2:
                ich_order = [3, 2]
            items = []
            for gi, ich in enumerate(ich_order):
                parts = groups[ich]
                for pi, (jt, slot, pa, pl, poff) in enumerate(parts):
                    items.append(
                        ("part", ich, jt, slot, pa, pl, poff,
                         pi == 0, pi == len(parts) - 1)
                    )
                items.append(("drain", ich, gi == len(ich_order) - 1))
            return items

        def emit_attv_item(i, item):
            g, h = units[i]
            if item[0] == "part":
                _, ich, jt, slot, pa, pl, poff, first, stop = item
                jl = _jl(jt)
                ysd = urec[i].setdefault("ps_y", {})
                if ich not in ysd:
                    ysd[ich] = psYp.tile([VW, 512], F32, name="ps_y", tag="ps_y")
                off = pa - ICH0[ich]
                nc.tensor.matmul(
                    ysd[ich][:, off : off + pl],
                    v_ones[:jl, jt, h, :],
                    urec[i]["pt"][:jl, slot, poff : poff + pl],
                    start=first,
                    stop=stop,
                    skip_group_check=True,
                )
            else:
                _, ich, is_final = item
                psy = urec[i]["ps_y"][ich]
                ilen = I_CHUNKS[ich][1]
                rc = phB.tile([1, 512], F32, name="rc", tag="rc", bufs=4)
                nc.vector.reciprocal_approx_fast(
                    out=rc[0:1, :ilen], in_=psy[0:1, :ilen]
                )
                rc_bc = phB.tile([D, 512], F32, name="rc_bc", tag="rc_bc", bufs=4)
                nc.gpsimd.partition_broadcast(rc_bc[:, :ilen], rc[0:1, :ilen])
                if is_final:
                    urec[i]["pending"] = (ich, psy, rc_bc)
                else:
                    emit_norm(i, ich, psy, rc_bc)

        def emit_norm(i, ich, psy, rc_bc):
            g, h = units[i]
            pof = D * (h % 2)
            ct = h // 2
            i0, ilen = I_CHUNKS[ich]
            nc.vector.tensor_tensor(
                yT[pof : pof + D, ct, i0 : i0 + ilen],
                psy[V0 : V0 + D, :ilen],
                rc_bc[:, :ilen],
                mybir.AluOpType.mult,
            )

        def emit_pending_norm(i):
            if "pending" in urec[i]:
                ich, psy, rc_bc = urec[i].pop("pending")
                emit_norm(i, ich, psy, rc_bc)

        # ---------- filler segments ----------
        f1 = deque()
        for ct in range(CT):
            if ct == 0:
                f1.append(lambda: emit_qk_chain(qT, wq_sb, bq_sb, 0, 1))
                for ich in (1, 2, 3):
                    f1.append(
                        lambda ich=ich: emit_qk_chain(kT, wk_sb, bk_sb, 0, ich)
                    )
                for it in range(4, NJT):
                    f1.append(lambda it=it: emit_v_chain(it))
            else:
                f1.append(lambda ct=ct: emit_qk_chain(qT, wq_sb, bq_sb, ct, 1))
                for ich in (1, 2, 3):
                    f1.append(
                        lambda ct=ct, ich=ich: emit_qk_chain(
                            kT, wk_sb, bk_sb, ct, ich
                        )
                    )
        f2 = deque()
        for ich in (2, 3):
            for ct in range(CT):
                f2.append(
                    lambda ct=ct, ich=ich: emit_qk_chain(qT, wq_sb, bq_sb, ct, ich)
                )
        f3 = deque(
            (lambda it=it, nch=nch: emit_outproj_chain(it, nch))
            for it in range(0, 4)
            for nch in range(2)
        )
        f4 = deque(
            (lambda it=it, nch=nch: emit_outproj_chain(it, nch))
            for it in range(4, 8)
            for nch in range(2)
        )
        segments = [(0, f1), (6, f2), (7, f3), (13, f4)]

        def pop_filler(block):
            for rel, dq in segments:
                if rel <= block and dq:
                    dq.popleft()()
                    return True
            return False

        QUOTA = [4, 4, 4, 3, 3, 3, 2, 2, 2, 2, 2, 2, 2, 2, 2, 2, 2, 2]

        # ---------- upfront: phase A for g0's needs ----------
        for ct in range(CT):
            emit_qk_chain(kT, wk_sb, bk_sb, ct, 0)
            emit_qk_chain(qT, wq_sb, bq_sb, ct, 0)
        for it in range(4):
            emit_v_chain(it)

        # ---------- main software-pipelined loop ----------
        for i, (g, h) in enumerate(units):
            if i == 6:
                while f1:
                    f1.popleft()()
            if i == 12:
                while f2:
                    f2.popleft()()
            if i >= 2:
                emit_pending_norm(i - 2)
            urec[i]["pt"] = phB.tile(
                [P, NJT, SLOT], F16, name="pt", tag="pt", bufs=3
            )
            slots = _unit_slots(g)
            chunks = []
            for jt, slot, a, mcl, tcl in slots:
                chunks.append((jt, slot, a, mcl, 0))
                if tcl:
                    chunks.append((jt, slot, a + 512, tcl, 512))
            urec[i]["chunks"] = chunks

            # exp-units: (fn, main_slot_done)
            eus = []
            if g == 0:
                eus.append((lambda s=slots: emit_pair(i, s[0], s[1]), 1))
                eus.append((lambda s=slots: emit_pair(i, s[2], s[3]), 3))
            else:
                for k in range(6):
                    eus.append(
                        (lambda s=slots, k=k: emit_pair(i, s[2 * k], s[2 * k + 1]),
                         2 * k + 1)
                    )
                eus.append((lambda s=slots: emit_single12(i, s[12]), 12))
                if g == 2:
                    eus.append((lambda s=slots: emit_tailpair(i, s[0], s[1]), -1))
                    eus.append((lambda s=slots: emit_tail8(i, s[8]), -1))

            av = attv_items(i - 1) if i >= 1 else []
            nE = len(eus)
            nwin = 1 if g == 0 else 3
            quota = QUOTA[i]
            pops = 0
            ai = 0
            next_w = 0
            for e_idx, (fn, sdone) in enumerate(eus):
                fn()
                while next_w < nwin and 4 * next_w + 3 <= sdone:
                    emit_mask_window(i, next_w)
                    next_w += 1
                if sdone == 12 and g >= 1:
                    emit_text_mask(i)
                tgt = (e_idx + 1) * len(av) // nE
                while ai < tgt:
                    emit_attv_item(i - 1, av[ai])
                    ai += 1
                if pops < quota and (e_idx + 1) * quota >= (pops + 1) * nE:
                    if pop_filler(i):
                        pops += 1
            while ai < len(av):
                emit_attv_item(i - 1, av[ai])
                ai += 1

        # ---------- tail ----------
        nu = len(units)
        emit_pending_norm(nu - 2)
        av = attv_items(nu - 1)
        for item in av:
            emit_attv_item(nu - 1, item)
            if item[0] == "drain" and item[1] == 3:
                # it12 out-proj needs only the ich3 norms (text rows), which
                # just completed - run it while the ich2 attv still streams
                for nch in range(2):
                    emit_outproj_chain(12, nch, tail=True)
        emit_pending_norm(nu - 1)
        for it in range(8, 12):
            for nch in range(2):
                emit_outproj_chain(it, nch, tail=True)

    nc.compile()
    return nc


def _build_mask_np(seg_starts, seg_ends):
    """True = masked. Mirrors reference._build_mask in numpy."""
    ML = 3 * T
    tril = np.tril(np.ones((T, T), dtype=bool))
    sl = np.tril(np.ones((T, T), dtype=bool), -1)
    m = np.zeros((L, L), dtype=bool)
    m[:ML, :ML] = True
    m[0:T, 0:T] = ~tril
    m[T : 2 * T, 0:T] = ~tril
    m[T : 2 * T, T : 2 * T] = ~sl
    m[T : 2 * T, 2 * T : 3 * T] = ~sl
    m[2 * T : 3 * T, 0:T] = ~tril
    m[2 * T : 3 * T, T : 2 * T] = ~tril
    m[2 * T : 3 * T, 2 * T : 3 * T] = ~sl
    m[:ML, ML:] = True
    frames = np.arange(T)[None, :, None]
    allowed = (frames >= seg_starts[:, None, :]) & (frames < seg_ends[:, None, :])
    mask = np.broadcast_to(m[None], (B, L, L)).copy()
    for row0, col_blocks in ((T, (0, 2, 3)), (2 * T, (1, 2, 3))):
        for j in col_blocks:
            c0 = ML + j * N
            mask[:, row0 : row0 + T, c0 : c0 + N] &= ~allowed
    return mask


def get_nc():
    global _NC
    if _NC is None:
        _NC = _build_program()
    return _NC


def _build_maskstk():
    r = np.arange(P)
    t1 = (r[:, None] <= r[None, :]).astype(np.float16)  # tril.T
    t2 = (r[:, None] < r[None, :]).astype(np.float16)  # strict
    stk = np.empty((P, 28, P), dtype=np.float16)
    s = 0
    for g in range(3):
        ndiag = 4 if g == 0 else 12
        for jt in _SLOT_JTS[g][:ndiag]:
            stk[:, s, :] = t1 if _grp_diag(g, jt) == "T1" else t2
            s += 1
    assert s == 28
    return stk.reshape(P, 28 * P)


def make_in_maps(x, Wq, bq, Wk, bk, Wv, bv, Wp, bp, seg_starts, seg_ends):
    mask = _build_mask_np(np.asarray(seg_starts), np.asarray(seg_ends))
    maskstk = _build_maskstk()
    in_maps = []
    for core in range(8):
        b, g = core // 2, core % 2
        gs = slice(g * G, (g + 1) * G)
        allowT = ~mask[b].T  # [j, i]
        maskTx = np.ascontiguousarray(
            allowT[1536:1568, 512:1536].astype(np.float16)
        )
        in_maps.append(
            {
                "xT": np.ascontiguousarray(x[b].T).astype(np.float16),
                "wqT": np.ascontiguousarray(Wq[gs, :].T).astype(np.float16),
                "wkT": np.ascontiguousarray(Wk[gs, :].T).astype(np.float16),
                "wvT": np.ascontiguousarray(Wv[gs, :].T).astype(np.float16),
                "wpT": np.ascontiguousarray(Wp[:, gs].T).astype(np.float16),
                "bqP": np.ascontiguousarray(bq[gs].reshape(CT, P).T),
                "bkP": np.ascontiguousarray(bk[gs].reshape(CT, P).T),
                "maskStk": maskstk,
                "maskTxt": maskTx,
            }
        )
    return in_maps


def kernel(x, Wq, bq, Wk, bk, Wv, bv, Wp, bp, seg_starts, seg_ends, T_motion=None,
           N=None, _trace=False, **_unused):
    x = np.asarray(x, np.float32)
    args = [np.asarray(a, np.float32) for a in (Wq, bq, Wk, bk, Wv, bv, Wp, bp)]
    Wq, bq, Wk, bk, Wv, bv, Wp, bp = args
    nc = get_nc()
    in_maps = make_in_maps(x, Wq, bq, Wk, bk, Wv, bv, Wp, bp, seg_starts, seg_ends)
    res = run_bass_kernel_spmd(nc, in_maps, core_ids=list(range(8)), trace=_trace)
    parts = [np.asarray(r["out_part"], np.float32) for r in res.results]
    # v-bias folds into the output bias exactly: att rows sum to 1, so
    # y = att@(v+bv) = att@v + bv, and (y+bv)@Wp.T = y@Wp.T + bv@Wp.T
    bp_eff = bp + bv @ Wp.T
    y = np.empty((B, L, C), np.float32)
    for b in range(B):
        y[b] = parts[2 * b] + parts[2 * b + 1] + bp_eff
    if _trace:
        kernel.last_results = res
    return y
